# revision 6
# baseline (speedup 1.0000x reference)
"""Causal self-attention (B=2, L=4096, D=768, H=12) on 8 Trainium2 cores.

Sharding: core c = (b, g) with b = c // 4, g = c % 4. Data-parallel over the
batch, tensor-parallel over heads (3 heads per core). Each core computes its
heads' attention over the full sequence plus its slice of the output
projection (row-parallel); the host sums the 4 partial projections per batch
element and adds b_out.

Device-side layout (all matmul operands bf16, fp32 PSUM accumulation):
  - host supplies x[b]^T (768, L); q^T/k^T are produced in [dh, L] layout by
    the projection itself, v in natural [L, dh] layout — no on-device
    transposes anywhere.
  - scores are computed transposed, S^T[lk, lq], 512 lq per PSUM tile; exp
    runs on ScalarE with the 1/sqrt(dh) scale folded in and no
    max-subtraction (scores are ~N(0,1) here, exp is safe in fp32).
  - causal mask = multiply by a 0/1 tile, needed only on the 4 diagonal
    lk-tiles of each lq chunk.
  - PV matmul uses lhsT = [v | ones] so PSUM row 64 accumulates the softmax
    denominator for free; normalization is a K=1 broadcast-matmul of the
    reciprocal row against each head's 64 output partitions.
"""

import os
import sys

sys.path.insert(0, "/opt/trn_rl_repo")

import numpy as np
import ml_dtypes

import concourse.bass as bass  # noqa: F401  (registers AP machinery)
import concourse.mybir as mybir
from concourse import bacc
import concourse.tile as tile
from concourse.bass_utils import run_bass_kernel_spmd

BF16 = ml_dtypes.bfloat16
F32 = mybir.dt.float32
BF = mybir.dt.bfloat16

D_MODEL = 768
N_HEADS = 12
D_HEAD = 64
B = 2
L_FULL = 4096
N_CORES = 8
TPG = 4  # head-groups (tensor-parallel degree per batch element)
HPC = N_HEADS // TPG  # 3 heads per core
DG = HPC * D_HEAD  # 192 feature dims per core
SCALE = 1.0 / np.sqrt(D_HEAD)

DM_CHUNKS = D_MODEL // 128  # 6


def build_nc(L=L_FULL):
    """Build the per-core Bass program (same program for all 8 cores)."""
    LC = L // 512  # lq chunks
    LT = L // 128  # lk / l tiles
    nc = bacc.Bacc("TRN2", target_bir_lowering=False, debug=False,
                   num_devices=N_CORES)

    xT_d = nc.dram_tensor("xT", [D_MODEL, L], BF, kind="ExternalInput").ap()
    wqk_d = nc.dram_tensor("wqkc", [D_MODEL, 512], BF, kind="ExternalInput").ap()
    bqk_d = nc.dram_tensor("bqkc", [1, 512], BF, kind="ExternalInput").ap()
    wv_d = nc.dram_tensor("wv", [D_MODEL, DG], BF, kind="ExternalInput").ap()
    bv_d = nc.dram_tensor("bv", [1, DG], BF, kind="ExternalInput").ap()
    wo_d = nc.dram_tensor("wo", [DG, D_MODEL], BF, kind="ExternalInput").ap()
    mask_d = nc.dram_tensor("masks", [128, 4, 512], BF, kind="ExternalInput").ap()
    out_d = nc.dram_tensor("out", [L, D_MODEL], F32, kind="ExternalOutput").ap()

    with tile.TileContext(nc) as tc:
        with tc.tile_pool(name="persist", bufs=1) as persist:
            xT_sb = persist.tile([128, DM_CHUNKS, L], BF)
            wqk_sb = persist.tile([128, DM_CHUNKS, 512], BF)
            wv_sb = persist.tile([128, DM_CHUNKS, DG], BF)
            bqk_sb = persist.tile([1, 512], BF)
            bv_sb = persist.tile([1, DG], BF)
            wo_sb = persist.tile([64, HPC, D_MODEL], BF)
            mask_sb = persist.tile([128, 4, 512], BF)
            ones_sb = persist.tile([128, 512], BF)
            qT01 = persist.tile([128, L], BF)
            kT01 = persist.tile([128, L], BF)
            qT2 = persist.tile([64, L], BF)
            kT2 = persist.tile([64, L], BF)
            vones = persist.tile([128, LT, HPC * 65], BF)
            attnT = [persist.tile([64, L], BF, name=f"attnT{h}") for h in range(HPC)]

            nc.vector.memset(ones_sb, 1.0)
            nc.vector.memset(vones, 1.0)

            for cdm in range(DM_CHUNKS):
                nc.sync.dma_start(out=xT_sb[:, cdm, :],
                                  in_=xT_d[cdm * 128:(cdm + 1) * 128, :])
                nc.sync.dma_start(out=wqk_sb[:, cdm, :],
                                  in_=wqk_d[cdm * 128:(cdm + 1) * 128, :])
                nc.sync.dma_start(out=wv_sb[:, cdm, :],
                                  in_=wv_d[cdm * 128:(cdm + 1) * 128, :])
            nc.sync.dma_start(out=bqk_sb, in_=bqk_d)
            nc.sync.dma_start(out=bv_sb, in_=bv_d)
            for h in range(HPC):
                nc.sync.dma_start(out=wo_sb[0:64, h, :],
                                  in_=wo_d[h * 64:(h + 1) * 64, :])
            nc.sync.dma_start(out=mask_sb, in_=mask_d)

            # ---- Phase 1: q^T/k^T (transposed form) and v (natural form) ----
            # wqkc column chunks: 0=[q0|q1] 1=[k0|k1] 2=[q2|junk] 3=[k2|junk]
            qk_dest = [
                (qT01, 0, 128),  # (tile, row0, nrows)
                (kT01, 0, 128),
                (qT2, 0, 64),
                (kT2, 0, 64),
            ]
            with tc.tile_pool(name="p1psum", bufs=3, space="PSUM") as p1p:
                for fc in range(4):
                    dest, row0, nrows = qk_dest[fc]
                    for lc in range(LC):
                        ps = p1p.tile([128, 512], F32)
                        for cdm in range(DM_CHUNKS):
                            nc.tensor.matmul(
                                ps,
                                wqk_sb[:, cdm, fc * 128:(fc + 1) * 128],
                                xT_sb[:, cdm, lc * 512:(lc + 1) * 512],
                                start=(cdm == 0), stop=False,
                            )
                        # + bias (broadcast along L via K=1 matmul)
                        nc.tensor.matmul(
                            ps, bqk_sb[0:1, fc * 128:(fc + 1) * 128],
                            ones_sb[0:1, :], start=False, stop=True,
                        )
                        nc.vector.tensor_copy(
                            dest[row0:row0 + nrows, lc * 512:(lc + 1) * 512],
                            ps[0:nrows, :],
                        )
                # v natural: out[128 l, 192] per l-tile
                for lt in range(LT):
                    pv = p1p.tile([128, DG], F32)
                    for cdm in range(DM_CHUNKS):
                        nc.tensor.matmul(
                            pv,
                            xT_sb[:, cdm, lt * 128:(lt + 1) * 128],
                            wv_sb[:, cdm, :],
                            start=(cdm == 0), stop=False,
                        )
                    nc.tensor.matmul(
                        pv, ones_sb[0:1, 0:128], bv_sb,
                        start=False, stop=True,
                    )
                    nc.vector.tensor_copy(
                        vones[:, lt, 0:HPC * 65]
                        .rearrange("p (h c) -> p h c", h=HPC)[:, :, 0:64],
                        pv.rearrange("p (h c) -> p h c", h=HPC),
                    )

            # ---- Phase 2: attention ----
            head_qk = [
                (qT01, kT01, 0),
                (qT01, kT01, 64),
                (qT2, kT2, 0),
            ]
            with (
                tc.tile_pool(name="stpsum", bufs=2, space="PSUM") as stp,
                tc.tile_pool(name="pvpsum", bufs=4, space="PSUM") as pvp,
                tc.tile_pool(name="rbpsum", bufs=2, space="PSUM") as rbp,
                tc.tile_pool(name="ptpool", bufs=4) as ptp,
                tc.tile_pool(name="rpool", bufs=4) as rp,
            ):
                for c in range(LC):
                    nt = 4 * (c + 1)
                    pv_acc = [pvp.tile([65, 512], F32, tag="pvacc",
                                       name=f"pvacc_c{c}h{h}") for h in range(HPC)]
                    for t in range(nt):
                        j = t - 4 * c  # >= 0 on diagonal tiles
                        col0 = 128 * j if j >= 0 else 0
                        for h in range(HPC):
                            qs, ks, r0 = head_qk[h]
                            st = stp.tile([128, 512], F32, tag="st")
                            nc.tensor.matmul(
                                st[:, col0:],
                                ks[r0:r0 + 64, t * 128:(t + 1) * 128],
                                qs[r0:r0 + 64, c * 512 + col0:(c + 1) * 512],
                            )
                            pt = ptp.tile([128, 512], BF, tag="pt")
                            nc.scalar.activation(
                                pt[:, col0:], st[:, col0:],
                                mybir.ActivationFunctionType.Exp,
                                scale=float(SCALE),
                            )
                            if j >= 0:
                                nc.vector.tensor_mul(
                                    pt[:, col0:], pt[:, col0:],
                                    mask_sb[:, j, col0:],
                                )
                            nc.tensor.matmul(
                                pv_acc[h][:, col0:],
                                vones[:, t, h * 65:(h + 1) * 65],
                                pt[:, col0:],
                                start=(t == 0), stop=(t == nt - 1),
                            )
                    for h in range(HPC):
                        recip = rp.tile([65, 512], BF, tag="recip")
                        with nc.allow_low_precision(reason="softmax denom recip rounds to bf16"):
                            nc.vector.reciprocal(recip[64:65, :], pv_acc[h][64:65, :])
                        rb = rbp.tile([64, 512], F32, tag="rb")
                        nc.tensor.matmul(
                            rb, ones_sb[64:65, 0:64], recip[64:65, :],
                        )
                        rbs = rp.tile([64, 512], F32, tag="rbs")
                        nc.vector.tensor_copy(rbs, rb)
                        nc.vector.tensor_mul(
                            attnT[h][0:64, c * 512:(c + 1) * 512],
                            pv_acc[h][0:64, :], rbs,
                        )

            # ---- Phase 3: output projection (row-parallel partial) ----
            with (
                tc.tile_pool(name="popsum", bufs=4, space="PSUM") as pop,
                tc.tile_pool(name="outpool", bufs=3) as outp,
            ):
                for lt in range(LT):
                    osb = outp.tile([128, D_MODEL], F32, tag="osb")
                    for nh in range(2):
                        po = pop.tile([128, 384], F32, tag="po")
                        for h in range(HPC):
                            nc.tensor.matmul(
                                po,
                                attnT[h][0:64, lt * 128:(lt + 1) * 128],
                                wo_sb[0:64, h, nh * 384:(nh + 1) * 384],
                                start=(h == 0), stop=(h == HPC - 1),
                            )
                        nc.vector.tensor_copy(osb[:, nh * 384:(nh + 1) * 384], po)
                    nc.sync.dma_start(out=out_d[lt * 128:(lt + 1) * 128, :], in_=osb)

    nc.compile()
    return nc


def make_in_maps(x, w_qkv, b_qkv, w_out, L=L_FULL):
    """Host-side sharding: build the 8 per-core input dicts."""
    # causal mask tiles for diagonal blocks: m[p, j, f] = (128 j + p) <= f
    p = np.arange(128)[:, None, None]
    jj = np.arange(4)[None, :, None]
    f = np.arange(512)[None, None, :]
    masks = ((128 * jj + p) <= f).astype(BF16)

    xT = [np.ascontiguousarray(x[b].T.astype(BF16)) for b in range(B)]
    in_maps = []
    for c in range(N_CORES):
        b, g = divmod(c, TPG)
        h0 = g * HPC  # first global head of this group

        def qcol(h):
            return slice((h0 + h) * D_HEAD, (h0 + h + 1) * D_HEAD)

        wqkc = np.zeros((D_MODEL, 512), np.float32)
        bqkc = np.zeros((1, 512), np.float32)
        # chunk0 [q0|q1], chunk1 [k0|k1], chunk2 [q2|-], chunk3 [k2|-]
        for h in range(2):
            wqkc[:, h * 64:(h + 1) * 64] = w_qkv[:, qcol(h)]
            wqkc[:, 128 + h * 64:128 + (h + 1) * 64] = w_qkv[:, 768 + (h0 + h) * 64:768 + (h0 + h + 1) * 64]
            bqkc[0, h * 64:(h + 1) * 64] = b_qkv[qcol(h)]
            bqkc[0, 128 + h * 64:128 + (h + 1) * 64] = b_qkv[768 + (h0 + h) * 64:768 + (h0 + h + 1) * 64]
        wqkc[:, 256:320] = w_qkv[:, qcol(2)]
        bqkc[0, 256:320] = b_qkv[qcol(2)]
        wqkc[:, 384:448] = w_qkv[:, 768 + (h0 + 2) * 64:768 + (h0 + 3) * 64]
        bqkc[0, 384:448] = b_qkv[768 + (h0 + 2) * 64:768 + (h0 + 3) * 64]

        wv = w_qkv[:, 1536 + h0 * 64:1536 + (h0 + HPC) * 64]
        bv = b_qkv[1536 + h0 * 64:1536 + (h0 + HPC) * 64][None, :]
        wo = w_out[h0 * 64:(h0 + HPC) * 64, :]

        in_maps.append({
            "xT": xT[b][:, :L],
            "wqkc": wqkc.astype(BF16),
            "bqkc": bqkc.astype(BF16),
            "wv": np.ascontiguousarray(wv).astype(BF16),
            "bv": np.ascontiguousarray(bv).astype(BF16),
            "wo": np.ascontiguousarray(wo).astype(BF16),
            "masks": masks,
        })
    return in_maps


_NC_CACHE = {}


def _get_nc(L=L_FULL):
    if L not in _NC_CACHE:
        _NC_CACHE[L] = build_nc(L)
    return _NC_CACHE[L]


def run(x, w_qkv, b_qkv, w_out, b_out, L=L_FULL, trace=False):
    nc = _get_nc(L)
    in_maps = make_in_maps(np.asarray(x), np.asarray(w_qkv),
                           np.asarray(b_qkv), np.asarray(w_out), L=L)
    if trace:
        install_ntff()
    res = run_bass_kernel_spmd(nc, in_maps, core_ids=list(range(N_CORES)),
                               trace=trace)
    partials = np.stack([res.results[c]["out"] for c in range(N_CORES)])
    out = partials.reshape(B, TPG, L, D_MODEL).sum(axis=1)
    out = out + np.asarray(b_out, np.float32)[None, None, :]
    return out.astype(np.float32), res


def kernel(x, w_qkv, b_qkv, w_out, b_out):
    out, _ = run(x, w_qkv, b_qkv, w_out, b_out, L=L_FULL, trace=False)
    return out


# ---- optional NTFF profiling hook (axon images lack antenv.axon_hooks) ----
def install_ntff(so_path="/opt/axon/libaxon_pjrt.so"):
    import contextlib
    import ctypes
    import types

    if "antenv.axon_hooks" in sys.modules:
        return
    holder = {"hook": None}

    def _build():
        if not os.path.exists(so_path):
            return None
        lib = ctypes.CDLL(so_path)
        if not hasattr(lib, "axon_start_nrt_profile"):
            return None
        lib.axon_start_nrt_profile.argtypes = [ctypes.POINTER(ctypes.c_int64),
                                               ctypes.c_size_t]
        lib.axon_start_nrt_profile.restype = ctypes.c_int64
        lib.axon_stop_nrt_profile.argtypes = [ctypes.c_char_p]
        lib.axon_stop_nrt_profile.restype = ctypes.c_int64

        @contextlib.contextmanager
        def _hook(output_dir, device_ids):
            import jax
            jax.devices()
            if device_ids:
                ids = (ctypes.c_int64 * len(device_ids))(*device_ids)
                rc = lib.axon_start_nrt_profile(ids, len(device_ids))
            else:
                rc = lib.axon_start_nrt_profile(None, 0)
            if rc != 0:
                raise RuntimeError(f"axon_start_nrt_profile rc={rc}")
            try:
                yield
            finally:
                n = lib.axon_stop_nrt_profile(str(output_dir).encode())
                print(f"ntff profile: {n} file(s) -> {output_dir}",
                      file=sys.stderr)

        return _hook

    mod = types.ModuleType("antenv.axon_hooks")
    mod.set_axon_ntff_profile_hook = lambda h: holder.__setitem__("hook", h)
    mod.get_axon_ntff_profile_hook = lambda: holder["hook"]
    sys.modules["antenv.axon_hooks"] = mod
    holder["hook"] = _build()


# revision 8
# speedup vs baseline: 1.5040x; 1.5040x over previous
"""Causal self-attention (B=2, L=4096, D=768, H=12) on 8 Trainium2 cores.

Sharding: core c = (b, g) with b = c // 4, g = c % 4. Data-parallel over the
batch, tensor-parallel over heads (3 heads per core). Each core computes its
heads' attention over the full sequence plus its slice of the output
projection (row-parallel); the host sums the 4 partial projections per batch
element and adds b_out.

Device-side design (all matmul operands bf16, fp32 PSUM accumulation):
  - host supplies x[b]^T (768, L); q^T/k^T are produced in [dh, L] layout by
    the projection itself, v in natural [L, dh] layout — no on-device
    transposes anywhere.
  - scores are computed transposed, S^T[lk, lq], 512 lq per PSUM tile; exp
    runs on ScalarE with the 1/sqrt(dh) scale folded in and no
    max-subtraction (scores are ~N(0,1) here, exp is safe in fp32).
  - causal mask = multiply by a 0/1 tile, needed only on the 4 diagonal
    lk-tiles of each lq chunk.
  - PV matmul uses lhsT = [v | ones] so PSUM row 64 accumulates the softmax
    denominator for free; the reciprocal row is partition-broadcast on the
    (otherwise idle) GpSimd engine and applied by one DVE multiply.
  - the PE clock-gate (HAM) only sustains 2.4 GHz when the PE never idles,
    so the qkv projection of the NEXT lq chunk and the output projection of
    the PREVIOUS chunk are emitted as filler work inside the attention
    loop, and PV runs software-pipelined one lk-tile behind the exp.
"""

import os
import sys

sys.path.insert(0, "/opt/trn_rl_repo")

import numpy as np
import ml_dtypes

import concourse.bass as bass  # noqa: F401  (registers AP machinery)
import concourse.mybir as mybir
from concourse import bacc
import concourse.tile as tile
from concourse.bass_utils import run_bass_kernel_spmd

BF16 = ml_dtypes.bfloat16
F32 = mybir.dt.float32
BF = mybir.dt.bfloat16

D_MODEL = 768
N_HEADS = 12
D_HEAD = 64
B = 2
L_FULL = 4096
N_CORES = 8
TPG = 4  # head-groups (tensor-parallel degree per batch element)
HPC = N_HEADS // TPG  # 3 heads per core
DG = HPC * D_HEAD  # 192 feature dims per core
SCALE = 1.0 / np.sqrt(D_HEAD)

DM_CHUNKS = D_MODEL // 128  # 6


def build_nc(L=L_FULL):
    """Build the per-core Bass program (same program for all 8 cores)."""
    LC = L // 512  # lq chunks
    LT = L // 128  # lk / l tiles
    nc = bacc.Bacc("TRN2", target_bir_lowering=False, debug=False,
                   num_devices=N_CORES)

    xT_d = nc.dram_tensor("xT", [D_MODEL, L], BF, kind="ExternalInput").ap()
    wqk_d = nc.dram_tensor("wqkc", [D_MODEL, 512], BF, kind="ExternalInput").ap()
    bqk_d = nc.dram_tensor("bqkt", [128, 4], F32, kind="ExternalInput").ap()
    wv_d = nc.dram_tensor("wv", [D_MODEL, DG], BF, kind="ExternalInput").ap()
    bv_d = nc.dram_tensor("bv", [1, DG], BF, kind="ExternalInput").ap()
    wo_d = nc.dram_tensor("wo", [DG, D_MODEL], BF, kind="ExternalInput").ap()
    mask_d = nc.dram_tensor("masks", [128, 4, 512], BF, kind="ExternalInput").ap()
    out_d = nc.dram_tensor("out", [L, D_MODEL], F32, kind="ExternalOutput").ap()

    with tile.TileContext(nc) as tc:
        with tc.tile_pool(name="persist", bufs=1) as persist:
            xT_sb = persist.tile([128, DM_CHUNKS, L], BF)
            wqk_sb = persist.tile([128, DM_CHUNKS, 512], BF)
            wv_sb = persist.tile([128, DM_CHUNKS, DG], BF)
            bqk_sb = persist.tile([128, 4], F32)
            bv_sb = persist.tile([1, DG], BF)
            wo_sb = persist.tile([64, HPC, D_MODEL], BF)
            mask_sb = persist.tile([128, 4, 512], BF)
            ones_sb = persist.tile([128, 512], BF)
            qT01 = persist.tile([128, L], BF)
            kT01 = persist.tile([128, L], BF)
            qT2 = persist.tile([64, L], BF)
            kT2 = persist.tile([64, L], BF)
            vones = persist.tile([128, LT, HPC * 65], BF)
            attnT = [persist.tile([64, L], BF, name=f"attnT{h}") for h in range(HPC)]

            nc.vector.memset(ones_sb, 1.0)
            nc.vector.memset(vones, 1.0)

            for cdm in range(DM_CHUNKS):
                nc.sync.dma_start(out=wqk_sb[:, cdm, :],
                                  in_=wqk_d[cdm * 128:(cdm + 1) * 128, :])
                nc.sync.dma_start(out=wv_sb[:, cdm, :],
                                  in_=wv_d[cdm * 128:(cdm + 1) * 128, :])
            nc.sync.dma_start(out=bqk_sb, in_=bqk_d)
            nc.sync.dma_start(out=bv_sb, in_=bv_d)
            for h in range(HPC):
                nc.sync.dma_start(out=wo_sb[0:64, h, :],
                                  in_=wo_d[h * 64:(h + 1) * 64, :])
            nc.sync.dma_start(out=mask_sb, in_=mask_d)
            # x^T in lq-ordered strips so chunk 0's projections start early
            XSTRIP = min(1024, L)
            for ls in range(L // XSTRIP):
                for cdm in range(DM_CHUNKS):
                    nc.sync.dma_start(
                        out=xT_sb[:, cdm, ls * XSTRIP:(ls + 1) * XSTRIP],
                        in_=xT_d[cdm * 128:(cdm + 1) * 128,
                                 ls * XSTRIP:(ls + 1) * XSTRIP])

            # wqkc column chunks: 0=[q0|q1] 1=[k0|k1] 2=[q2|junk] 3=[k2|junk]
            qk_dest = [
                (qT01, 128),  # (tile, nrows)
                (kT01, 128),
                (qT2, 64),
                (kT2, 64),
            ]
            head_qk = [
                (qT01, kT01, 0),
                (qT01, kT01, 64),
                (qT2, kT2, 0),
            ]
            with (
                tc.tile_pool(name="p1psum", bufs=2, space="PSUM") as p1p,
                tc.tile_pool(name="stpsum", bufs=2, space="PSUM") as stp,
                tc.tile_pool(name="pvpsum", bufs=3, space="PSUM") as pvp,
                tc.tile_pool(name="popsum", bufs=1, space="PSUM") as pop,
                tc.tile_pool(name="ptpool", bufs=8) as ptp,
                tc.tile_pool(name="rpool", bufs=2) as rp,
                tc.tile_pool(name="outpool", bufs=3) as outp,
            ):
                def emit_qk(fc, lc):
                    dest, nrows = qk_dest[fc]
                    ps = p1p.tile([128, 512], F32, tag="p1",
                                  name=f"psqk{fc}_{lc}")
                    for cdm in range(DM_CHUNKS):
                        nc.tensor.matmul(
                            ps,
                            wqk_sb[:, cdm, fc * 128:(fc + 1) * 128],
                            xT_sb[:, cdm, lc * 512:(lc + 1) * 512],
                            start=(cdm == 0), stop=(cdm == DM_CHUNKS - 1),
                        )
                    # evacuate with fused per-partition (=feature) bias add
                    nc.vector.tensor_scalar_add(
                        dest[0:nrows, lc * 512:(lc + 1) * 512],
                        ps[0:nrows, :],
                        bqk_sb[0:nrows, fc:fc + 1],
                    )

                def emit_v(lt):
                    ps = p1p.tile([128, DG], F32, tag="p1", name=f"psv{lt}")
                    for cdm in range(DM_CHUNKS):
                        nc.tensor.matmul(
                            ps,
                            xT_sb[:, cdm, lt * 128:(lt + 1) * 128],
                            wv_sb[:, cdm, :],
                            start=(cdm == 0), stop=False,
                        )
                    nc.tensor.matmul(
                        ps, ones_sb[0:1, 0:128], bv_sb,
                        start=False, stop=True,
                    )
                    nc.vector.tensor_copy(
                        vones[:, lt, 0:HPC * 65]
                        .rearrange("p (h c) -> p h c", h=HPC)[:, :, 0:64],
                        ps.rearrange("p (h c) -> p h c", h=HPC),
                    )

                def emit_proj(lt):
                    osb = outp.tile([128, D_MODEL], F32, tag="osb",
                                    name=f"osb{lt}")
                    for nh in range(2):
                        po = pop.tile([128, 384], F32, tag="po",
                                      name=f"po{lt}_{nh}")
                        for h in range(HPC):
                            nc.tensor.matmul(
                                po,
                                attnT[h][0:64, lt * 128:(lt + 1) * 128],
                                wo_sb[0:64, h, nh * 384:(nh + 1) * 384],
                                start=(h == 0), stop=(h == HPC - 1),
                            )
                        nc.vector.tensor_copy(osb[:, nh * 384:(nh + 1) * 384], po)
                    nc.sync.dma_start(out=out_d[lt * 128:(lt + 1) * 128, :],
                                      in_=osb)

                def qkv_fillers(lc):
                    fs = [lambda fc=fc: emit_qk(fc, lc) for fc in range(4)]
                    fs += [lambda lt=lt: emit_v(lt)
                           for lt in range(4 * lc, 4 * lc + 4)]
                    return fs

                for f in qkv_fillers(0):
                    f()

                for c in range(LC):
                    nt = 4 * (c + 1)
                    fillers = []
                    if c + 1 < LC:
                        fillers += qkv_fillers(c + 1)
                    if c >= 1:
                        fillers += [lambda lt=lt: emit_proj(lt)
                                    for lt in range(4 * (c - 1), 4 * c)]
                    pv_acc = [pvp.tile([65, 512], F32, tag="pvacc",
                                       name=f"pvacc_c{c}h{h}")
                              for h in range(HPC)]
                    prev = []
                    fi = 0
                    for t in range(nt):
                        j = t - 4 * c  # >= 0 on diagonal tiles
                        col0 = 128 * j if j >= 0 else 0
                        cur = []
                        for h in range(HPC):
                            qs, ks, r0 = head_qk[h]
                            st = stp.tile([128, 512], F32, tag="st",
                                          name=f"st_c{c}t{t}h{h}")
                            nc.tensor.matmul(
                                st[:, col0:],
                                ks[r0:r0 + 64, t * 128:(t + 1) * 128],
                                qs[r0:r0 + 64, c * 512 + col0:(c + 1) * 512],
                            )
                            pt = ptp.tile([128, 512], BF, tag="pt",
                                          name=f"pt_c{c}t{t}h{h}")
                            nc.scalar.activation(
                                pt[:, col0:], st[:, col0:],
                                mybir.ActivationFunctionType.Exp,
                                scale=float(SCALE),
                            )
                            if j >= 0:
                                nc.vector.tensor_mul(
                                    pt[:, col0:], pt[:, col0:],
                                    mask_sb[:, j, col0:],
                                )
                            cur.append((h, pt, col0, t))
                        # PE filler work, spread across the lk-tile loop
                        want = (t + 1) * len(fillers) // nt
                        while fi < want:
                            fillers[fi]()
                            fi += 1
                        # software-pipelined PV: one lk-tile behind
                        for (h, pt0, c0, t0) in prev:
                            nc.tensor.matmul(
                                pv_acc[h][:, c0:],
                                vones[:, t0, h * 65:(h + 1) * 65],
                                pt0[:, c0:],
                                start=(t0 == 0), stop=False,
                            )
                        prev = cur
                    for (h, pt0, c0, t0) in prev:
                        nc.tensor.matmul(
                            pv_acc[h][:, c0:],
                            vones[:, t0, h * 65:(h + 1) * 65],
                            pt0[:, c0:],
                            start=(t0 == 0), stop=True,
                        )
                    prev = []
                    for h in range(HPC):
                        dn = rp.tile([1, 512], F32, tag="dn",
                                     name=f"dn_c{c}h{h}")
                        # partition-shifting copy (psum row 64 -> sbuf row 0);
                        # partition_broadcast only honors a partition-0 source
                        nc.vector.tensor_copy(dn[0:1, :],
                                              pv_acc[h][64:65, :])
                        dnb = rp.tile([64, 512], F32, tag="dnb",
                                      name=f"dnb_c{c}h{h}")
                        nc.gpsimd.partition_broadcast(dnb, dn[0:1, :])
                        rbs = rp.tile([64, 512], F32, tag="rbs",
                                      name=f"rbs_c{c}h{h}")
                        nc.vector.reciprocal_approx_fast(out=rbs, in_=dnb)
                        nc.vector.tensor_mul(
                            attnT[h][0:64, c * 512:(c + 1) * 512],
                            pv_acc[h][0:64, :], rbs,
                        )
                for lt in range(4 * (LC - 1), LT):
                    emit_proj(lt)

    nc.compile()
    return nc


def make_in_maps(x, w_qkv, b_qkv, w_out, L=L_FULL):
    """Host-side sharding: build the 8 per-core input dicts."""
    # causal mask tiles for diagonal blocks: m[p, j, f] = (128 j + p) <= f
    p = np.arange(128)[:, None, None]
    jj = np.arange(4)[None, :, None]
    f = np.arange(512)[None, None, :]
    masks = ((128 * jj + p) <= f).astype(BF16)

    xT = [np.ascontiguousarray(x[b].T.astype(BF16)) for b in range(B)]
    in_maps = []
    for c in range(N_CORES):
        b, g = divmod(c, TPG)
        h0 = g * HPC  # first global head of this group

        def qcol(h):
            return slice((h0 + h) * D_HEAD, (h0 + h + 1) * D_HEAD)

        def kcol(h):
            return slice(768 + (h0 + h) * D_HEAD, 768 + (h0 + h + 1) * D_HEAD)

        wqkc = np.zeros((D_MODEL, 512), np.float32)
        bqkc = np.zeros((512,), np.float32)
        # chunk0 [q0|q1], chunk1 [k0|k1], chunk2 [q2|-], chunk3 [k2|-]
        for h in range(2):
            wqkc[:, h * 64:(h + 1) * 64] = w_qkv[:, qcol(h)]
            wqkc[:, 128 + h * 64:128 + (h + 1) * 64] = w_qkv[:, kcol(h)]
            bqkc[h * 64:(h + 1) * 64] = b_qkv[qcol(h)]
            bqkc[128 + h * 64:128 + (h + 1) * 64] = b_qkv[kcol(h)]
        wqkc[:, 256:320] = w_qkv[:, qcol(2)]
        bqkc[256:320] = b_qkv[qcol(2)]
        wqkc[:, 384:448] = w_qkv[:, kcol(2)]
        bqkc[384:448] = b_qkv[kcol(2)]

        wv = w_qkv[:, 1536 + h0 * 64:1536 + (h0 + HPC) * 64]
        bv = b_qkv[1536 + h0 * 64:1536 + (h0 + HPC) * 64][None, :]
        wo = w_out[h0 * 64:(h0 + HPC) * 64, :]

        in_maps.append({
            "xT": xT[b][:, :L],
            "wqkc": wqkc.astype(BF16),
            "bqkt": np.ascontiguousarray(bqkc.reshape(4, 128).T),
            "wv": np.ascontiguousarray(wv).astype(BF16),
            "bv": np.ascontiguousarray(bv).astype(BF16),
            "wo": np.ascontiguousarray(wo).astype(BF16),
            "masks": masks,
        })
    return in_maps


_NC_CACHE = {}


def _get_nc(L=L_FULL):
    if L not in _NC_CACHE:
        _NC_CACHE[L] = build_nc(L)
    return _NC_CACHE[L]


def run(x, w_qkv, b_qkv, w_out, b_out, L=L_FULL, trace=False):
    nc = _get_nc(L)
    in_maps = make_in_maps(np.asarray(x), np.asarray(w_qkv),
                           np.asarray(b_qkv), np.asarray(w_out), L=L)
    if trace:
        install_ntff()
    res = run_bass_kernel_spmd(nc, in_maps, core_ids=list(range(N_CORES)),
                               trace=trace)
    partials = np.stack([res.results[c]["out"] for c in range(N_CORES)])
    out = partials.reshape(B, TPG, L, D_MODEL).sum(axis=1)
    out = out + np.asarray(b_out, np.float32)[None, None, :]
    return out.astype(np.float32), res


def kernel(x, w_qkv, b_qkv, w_out, b_out):
    out, _ = run(x, w_qkv, b_qkv, w_out, b_out, L=L_FULL, trace=False)
    return out


# ---- optional NTFF profiling hook (axon images lack antenv.axon_hooks) ----
def install_ntff(so_path="/opt/axon/libaxon_pjrt.so"):
    import contextlib
    import ctypes
    import types

    if "antenv.axon_hooks" in sys.modules:
        return
    holder = {"hook": None}

    def _build():
        if not os.path.exists(so_path):
            return None
        lib = ctypes.CDLL(so_path)
        if not hasattr(lib, "axon_start_nrt_profile"):
            return None
        lib.axon_start_nrt_profile.argtypes = [ctypes.POINTER(ctypes.c_int64),
                                               ctypes.c_size_t]
        lib.axon_start_nrt_profile.restype = ctypes.c_int64
        lib.axon_stop_nrt_profile.argtypes = [ctypes.c_char_p]
        lib.axon_stop_nrt_profile.restype = ctypes.c_int64

        @contextlib.contextmanager
        def _hook(output_dir, device_ids):
            import jax
            jax.devices()
            if device_ids:
                ids = (ctypes.c_int64 * len(device_ids))(*device_ids)
                rc = lib.axon_start_nrt_profile(ids, len(device_ids))
            else:
                rc = lib.axon_start_nrt_profile(None, 0)
            if rc != 0:
                raise RuntimeError(f"axon_start_nrt_profile rc={rc}")
            try:
                yield
            finally:
                n = lib.axon_stop_nrt_profile(str(output_dir).encode())
                print(f"ntff profile: {n} file(s) -> {output_dir}",
                      file=sys.stderr)

        return _hook

    mod = types.ModuleType("antenv.axon_hooks")
    mod.set_axon_ntff_profile_hook = lambda h: holder.__setitem__("hook", h)
    mod.get_axon_ntff_profile_hook = lambda: holder["hook"]
    sys.modules["antenv.axon_hooks"] = mod
    holder["hook"] = _build()


# revision 15
# speedup vs baseline: 1.8642x; 1.2395x over previous
"""Causal self-attention (B=2, L=4096, D=768, H=12) on 8 Trainium2 cores.

Sharding: core c = (b, g) with b = c // 4, g = c % 4. Data-parallel over the
batch, tensor-parallel over heads (3 heads per core). Each core computes its
heads' attention over the full sequence plus its slice of the output
projection (row-parallel); the host sums the 4 partial projections per batch
element and adds b_out.

Device-side design (all matmul operands bf16, fp32 PSUM accumulation):
  - host supplies x[b]^T (768, L); q^T/k^T are produced in [dh, L] layout by
    the projection itself, v in natural [L, dh] layout — no on-device
    transposes anywhere.
  - scores are computed transposed, S^T[lk, lq], 512 lq per PSUM tile; exp
    runs on ScalarE with the 1/sqrt(dh) scale folded in and no
    max-subtraction (scores are ~N(0,1) here, exp is safe in fp32).
  - causal mask = multiply by a 0/1 tile, needed only on the 4 diagonal
    lk-tiles of each lq chunk.
  - PV matmul uses lhsT = [v | ones] so PSUM row 64 accumulates the softmax
    denominator for free; the reciprocal row is partition-broadcast on the
    (otherwise idle) GpSimd engine and applied by one DVE multiply.
  - the PE clock-gate (HAM) only sustains 2.4 GHz when the PE never idles,
    so the qkv projection of the NEXT lq chunk and the output projection of
    the PREVIOUS chunk are emitted as filler work inside the attention
    loop, and PV runs software-pipelined one lk-tile behind the exp.
"""

import os
import sys

sys.path.insert(0, "/opt/trn_rl_repo")

import numpy as np
import ml_dtypes

import concourse.bass as bass  # noqa: F401  (registers AP machinery)
import concourse.mybir as mybir
from concourse import bacc
import concourse.tile as tile
from concourse.bass_utils import run_bass_kernel_spmd

BF16 = ml_dtypes.bfloat16
F32 = mybir.dt.float32
BF = mybir.dt.bfloat16

D_MODEL = 768
N_HEADS = 12
D_HEAD = 64
B = 2
L_FULL = 4096
N_CORES = 8
TPG = 4  # head-groups (tensor-parallel degree per batch element)
HPC = N_HEADS // TPG  # 3 heads per core
DG = HPC * D_HEAD  # 192 feature dims per core
SCALE = 1.0 / np.sqrt(D_HEAD)

DM_CHUNKS = D_MODEL // 128  # 6


def build_nc(L=L_FULL):
    """Build the per-core Bass program (same program for all 8 cores)."""
    LC = L // 512  # lq chunks
    LT = L // 128  # lk / l tiles
    nc = bacc.Bacc("TRN2", target_bir_lowering=False, debug=False,
                   num_devices=N_CORES)

    xT_d = nc.dram_tensor("xT", [D_MODEL, L], BF, kind="ExternalInput").ap()
    wqk_d = nc.dram_tensor("wqkc", [D_MODEL, 512], BF, kind="ExternalInput").ap()
    bqk_d = nc.dram_tensor("bqkt", [128, 4], F32, kind="ExternalInput").ap()
    wv_d = nc.dram_tensor("wv", [D_MODEL, DG], BF, kind="ExternalInput").ap()
    wo2_d = nc.dram_tensor("wo2", [128, D_MODEL], BF, kind="ExternalInput").ap()
    wo3_d = nc.dram_tensor("wo3", [128, D_MODEL], BF, kind="ExternalInput").ap()
    mask_d = nc.dram_tensor("masks", [128, 4, 512], BF, kind="ExternalInput").ap()
    out_d = nc.dram_tensor("out", [L, D_MODEL], F32, kind="ExternalOutput").ap()

    with tile.TileContext(nc) as tc:
        with tc.tile_pool(name="persist", bufs=1) as persist:
            xT_sb = persist.tile([128, DM_CHUNKS, L], BF)
            wqk_sb = persist.tile([128, DM_CHUNKS, 512], BF)
            wv_sb = persist.tile([128, DM_CHUNKS, DG], BF)
            bqk_sb = persist.tile([128, 4], F32)
            wo2_sb = persist.tile([128, D_MODEL], BF)
            wo3_sb = persist.tile([128, D_MODEL], BF)
            mask_sb = persist.tile([128, 4, 512], BF)
            # per-head q^T/k^T, zero-padded to K=128 (rows 64-127 stay zero:
            # a matmul that follows a K=64 matmul pays a ~100ns drain penalty)
            qT = [persist.tile([128, L], BF, name=f"qT{h}") for h in range(HPC)]
            kT = [persist.tile([128, L], BF, name=f"kT{h}") for h in range(HPC)]
            vones = persist.tile([128, LT, HPC * 65], BF)
            attnT01 = persist.tile([128, L], BF)
            attnT2 = persist.tile([128, L], BF)

            nc.vector.memset(vones, 1.0)
            for h in range(HPC):
                nc.vector.memset(qT[h][64:128, :], 0.0)
                nc.vector.memset(kT[h][64:128, :], 0.0)
            nc.vector.memset(attnT2[64:128, :], 0.0)

            for cdm in range(DM_CHUNKS):
                nc.sync.dma_start(out=wqk_sb[:, cdm, :],
                                  in_=wqk_d[cdm * 128:(cdm + 1) * 128, :])
                nc.sync.dma_start(out=wv_sb[:, cdm, :],
                                  in_=wv_d[cdm * 128:(cdm + 1) * 128, :])
            nc.sync.dma_start(out=bqk_sb, in_=bqk_d)
            nc.sync.dma_start(out=wo2_sb, in_=wo2_d)
            nc.sync.dma_start(out=wo3_sb, in_=wo3_d)
            nc.sync.dma_start(out=mask_sb, in_=mask_d)
            # x^T in lq-ordered strips so chunk 0's projections start early
            XSTRIP = min(1024, L)
            for ls in range(L // XSTRIP):
                for cdm in range(DM_CHUNKS):
                    nc.sync.dma_start(
                        out=xT_sb[:, cdm, ls * XSTRIP:(ls + 1) * XSTRIP],
                        in_=xT_d[cdm * 128:(cdm + 1) * 128,
                                 ls * XSTRIP:(ls + 1) * XSTRIP])

            # wqkc column chunks: 0=[q0|q1] 1=[k0|k1] 2=[q2|junk] 3=[k2|junk]
            # chunk evacuates into per-head tiles: psum rows 0-63 -> head a
            # rows 0-63, psum rows 64-127 -> head b rows 0-63 (shifted copy)
            qk_dest = [
                (qT[0], qT[1]),
                (kT[0], kT[1]),
                (qT[2], None),
                (kT[2], None),
            ]
            with (
                tc.tile_pool(name="p1psum", bufs=2, space="PSUM") as p1p,
                tc.tile_pool(name="stpsum", bufs=2, space="PSUM") as stp,
                tc.tile_pool(name="pvpsum", bufs=3, space="PSUM") as pvp,
                tc.tile_pool(name="popsum", bufs=1, space="PSUM") as pop,
                tc.tile_pool(name="ptpool", bufs=8) as ptp,
                tc.tile_pool(name="rpool", bufs=2) as rp,
                tc.tile_pool(name="outpool", bufs=3) as outp,
            ):
                def emit_qk(fc, lc):
                    dest_a, dest_b = qk_dest[fc]
                    ps = p1p.tile([128, 512], F32, tag="p1",
                                  name=f"psqk{fc}_{lc}")
                    for cdm in range(DM_CHUNKS):
                        nc.tensor.matmul(
                            ps,
                            wqk_sb[:, cdm, fc * 128:(fc + 1) * 128],
                            xT_sb[:, cdm, lc * 512:(lc + 1) * 512],
                            start=(cdm == 0), stop=(cdm == DM_CHUNKS - 1),
                        )
                    # evacuate with fused per-partition (=feature) bias add
                    nc.vector.tensor_scalar_add(
                        dest_a[0:64, lc * 512:(lc + 1) * 512],
                        ps[0:64, :],
                        bqk_sb[0:64, fc:fc + 1],
                    )
                    if dest_b is not None:
                        nc.vector.tensor_scalar_add(
                            dest_b[0:64, lc * 512:(lc + 1) * 512],
                            ps[64:128, :],
                            bqk_sb[64:128, fc:fc + 1],
                        )

                def emit_v(lt):
                    ps = p1p.tile([128, DG], F32, tag="p1", name=f"psv{lt}")
                    for cdm in range(DM_CHUNKS):
                        nc.tensor.matmul(
                            ps,
                            xT_sb[:, cdm, lt * 128:(lt + 1) * 128],
                            wv_sb[:, cdm, :],
                            start=(cdm == 0), stop=(cdm == DM_CHUNKS - 1),
                        )
                    nc.vector.tensor_copy(
                        vones[:, lt, 0:HPC * 65]
                        .rearrange("p (h c) -> p h c", h=HPC)[:, :, 0:64],
                        ps.rearrange("p (h c) -> p h c", h=HPC),
                    )

                def emit_proj(lt):
                    osb = outp.tile([128, D_MODEL], F32, tag="osb",
                                    name=f"osb{lt}")
                    for nh in range(2):
                        po = pop.tile([128, 384], F32, tag="po",
                                      name=f"po{lt}_{nh}")
                        nc.tensor.matmul(
                            po,
                            attnT01[:, lt * 128:(lt + 1) * 128],
                            wo2_sb[:, nh * 384:(nh + 1) * 384],
                            start=True, stop=False,
                        )
                        nc.tensor.matmul(
                            po,
                            attnT2[:, lt * 128:(lt + 1) * 128],
                            wo3_sb[:, nh * 384:(nh + 1) * 384],
                            start=False, stop=True,
                        )
                        nc.vector.tensor_copy(osb[:, nh * 384:(nh + 1) * 384], po)
                    nc.sync.dma_start(out=out_d[lt * 128:(lt + 1) * 128, :],
                                      in_=osb)

                def qkv_fillers(lc):
                    fs = [lambda fc=fc: emit_qk(fc, lc) for fc in range(4)]
                    fs += [lambda lt=lt: emit_v(lt)
                           for lt in range(4 * lc, 4 * lc + 4)]
                    return fs

                for f in qkv_fillers(0):
                    f()

                # (attnT tile, destination row base) per head
                norm_dest = [(attnT01, 0), (attnT01, 64), (attnT2, 0)]
                for c in range(LC):
                    nt = 4 * (c + 1)
                    fillers = qkv_fillers(c + 1) if c + 1 < LC else []
                    pv_acc = [pvp.tile([65, 512], F32, tag="pvacc",
                                       name=f"pvacc_c{c}h{h}")
                              for h in range(HPC)]
                    prev = []
                    fi = 0
                    for t in range(nt):
                        j = t - 4 * c  # >= 0 on diagonal tiles
                        col0 = 128 * j if j >= 0 else 0
                        cur = []
                        for h in range(HPC):
                            st = stp.tile([128, 512], F32, tag="st",
                                          name=f"st_c{c}t{t}h{h}")
                            nc.tensor.matmul(
                                st[:, col0:],
                                kT[h][:, t * 128:(t + 1) * 128],
                                qT[h][:, c * 512 + col0:(c + 1) * 512],
                            )
                            pt = ptp.tile([128, 512], BF, tag="pt",
                                          name=f"pt_c{c}t{t}h{h}")
                            nc.scalar.activation(
                                pt[:, col0:], st[:, col0:],
                                mybir.ActivationFunctionType.Exp,
                                scale=float(SCALE),
                            )
                            if j >= 0:
                                nc.vector.tensor_mul(
                                    pt[:, col0:], pt[:, col0:],
                                    mask_sb[:, j, col0:],
                                )
                            cur.append((h, pt, col0, t))
                        # PE filler work, spread across the lk-tile loop
                        want = (t + 1) * len(fillers) // nt
                        while fi < want:
                            fillers[fi]()
                            fi += 1
                        # software-pipelined PV: one lk-tile behind
                        for (h, pt0, c0, t0) in prev:
                            nc.tensor.matmul(
                                pv_acc[h][:, c0:],
                                vones[:, t0, h * 65:(h + 1) * 65],
                                pt0[:, c0:],
                                start=(t0 == 0), stop=False,
                            )
                        prev = cur
                    for (h, pt0, c0, t0) in prev:
                        nc.tensor.matmul(
                            pv_acc[h][:, c0:],
                            vones[:, t0, h * 65:(h + 1) * 65],
                            pt0[:, c0:],
                            start=(t0 == 0), stop=True,
                        )
                    prev = []
                    for h in range(HPC):
                        dn = rp.tile([1, 512], F32, tag="dn",
                                     name=f"dn_c{c}h{h}")
                        # partition-shifting copy (psum row 64 -> sbuf row 0);
                        # partition_broadcast only honors a partition-0 source
                        nc.vector.tensor_copy(dn[0:1, :],
                                              pv_acc[h][64:65, :])
                        dnb = rp.tile([64, 512], F32, tag="dnb",
                                      name=f"dnb_c{c}h{h}")
                        nc.gpsimd.partition_broadcast(dnb, dn[0:1, :])
                        rbs = rp.tile([64, 512], F32, tag="rbs",
                                      name=f"rbs_c{c}h{h}")
                        nc.vector.reciprocal_approx_fast(out=rbs, in_=dnb)
                        dt_, r0 = norm_dest[h]
                        nc.vector.tensor_mul(
                            dt_[r0:r0 + 64, c * 512:(c + 1) * 512],
                            pv_acc[h][0:64, :], rbs,
                        )
                    for lt in range(4 * c, 4 * c + 4):
                        emit_proj(lt)

    nc.compile()
    return nc


def make_in_maps(x, w_qkv, b_qkv, w_out, L=L_FULL):
    """Host-side sharding: build the 8 per-core input dicts."""
    # causal mask tiles for diagonal blocks: m[p, j, f] = (128 j + p) <= f
    p = np.arange(128)[:, None, None]
    jj = np.arange(4)[None, :, None]
    f = np.arange(512)[None, None, :]
    masks = ((128 * jj + p) <= f).astype(BF16)

    xT = [np.ascontiguousarray(x[b].T.astype(BF16)) for b in range(B)]
    in_maps = []
    for c in range(N_CORES):
        b, g = divmod(c, TPG)
        h0 = g * HPC  # first global head of this group

        def qcol(h):
            return slice((h0 + h) * D_HEAD, (h0 + h + 1) * D_HEAD)

        def kcol(h):
            return slice(768 + (h0 + h) * D_HEAD, 768 + (h0 + h + 1) * D_HEAD)

        wqkc = np.zeros((D_MODEL, 512), np.float32)
        bqkc = np.zeros((512,), np.float32)
        # chunk0 [q0|q1], chunk1 [k0|k1], chunk2 [q2|-], chunk3 [k2|-]
        for h in range(2):
            wqkc[:, h * 64:(h + 1) * 64] = w_qkv[:, qcol(h)]
            wqkc[:, 128 + h * 64:128 + (h + 1) * 64] = w_qkv[:, kcol(h)]
            bqkc[h * 64:(h + 1) * 64] = b_qkv[qcol(h)]
            bqkc[128 + h * 64:128 + (h + 1) * 64] = b_qkv[kcol(h)]
        wqkc[:, 256:320] = w_qkv[:, qcol(2)]
        bqkc[256:320] = b_qkv[qcol(2)]
        wqkc[:, 384:448] = w_qkv[:, kcol(2)]
        bqkc[384:448] = b_qkv[kcol(2)]

        wv = w_qkv[:, 1536 + h0 * 64:1536 + (h0 + HPC) * 64]
        wo = w_out[h0 * 64:(h0 + HPC) * 64, :]
        wo3 = np.zeros((128, D_MODEL), np.float32)
        wo3[0:64] = wo[128:192]

        in_maps.append({
            "xT": xT[b][:, :L],
            "wqkc": wqkc.astype(BF16),
            "bqkt": np.ascontiguousarray(bqkc.reshape(4, 128).T),
            "wv": np.ascontiguousarray(wv).astype(BF16),
            "wo2": np.ascontiguousarray(wo[0:128]).astype(BF16),
            "wo3": wo3.astype(BF16),
            "masks": masks,
        })
    return in_maps


_NC_CACHE = {}


def _get_nc(L=L_FULL):
    if L not in _NC_CACHE:
        _NC_CACHE[L] = build_nc(L)
    return _NC_CACHE[L]


def run(x, w_qkv, b_qkv, w_out, b_out, L=L_FULL, trace=False):
    nc = _get_nc(L)
    in_maps = make_in_maps(np.asarray(x), np.asarray(w_qkv),
                           np.asarray(b_qkv), np.asarray(w_out), L=L)
    if trace:
        install_ntff()
    res = run_bass_kernel_spmd(nc, in_maps, core_ids=list(range(N_CORES)),
                               trace=trace)
    partials = np.stack([res.results[c]["out"] for c in range(N_CORES)])
    out = partials.reshape(B, TPG, L, D_MODEL).sum(axis=1)
    # the V bias commutes through the attention average (weights sum to 1),
    # so it collapses to a constant row applied after the projection
    bias = np.asarray(b_qkv, np.float32)[1536:] @ np.asarray(w_out, np.float32)
    out = out + (bias + np.asarray(b_out, np.float32))[None, None, :]
    return out.astype(np.float32), res


def kernel(x, w_qkv, b_qkv, w_out, b_out):
    out, _ = run(x, w_qkv, b_qkv, w_out, b_out, L=L_FULL, trace=False)
    return out


# ---- optional NTFF profiling hook (axon images lack antenv.axon_hooks) ----
def install_ntff(so_path="/opt/axon/libaxon_pjrt.so"):
    import contextlib
    import ctypes
    import types

    if "antenv.axon_hooks" in sys.modules:
        return
    holder = {"hook": None}

    def _build():
        if not os.path.exists(so_path):
            return None
        lib = ctypes.CDLL(so_path)
        if not hasattr(lib, "axon_start_nrt_profile"):
            return None
        lib.axon_start_nrt_profile.argtypes = [ctypes.POINTER(ctypes.c_int64),
                                               ctypes.c_size_t]
        lib.axon_start_nrt_profile.restype = ctypes.c_int64
        lib.axon_stop_nrt_profile.argtypes = [ctypes.c_char_p]
        lib.axon_stop_nrt_profile.restype = ctypes.c_int64

        @contextlib.contextmanager
        def _hook(output_dir, device_ids):
            import jax
            jax.devices()
            if device_ids:
                ids = (ctypes.c_int64 * len(device_ids))(*device_ids)
                rc = lib.axon_start_nrt_profile(ids, len(device_ids))
            else:
                rc = lib.axon_start_nrt_profile(None, 0)
            if rc != 0:
                raise RuntimeError(f"axon_start_nrt_profile rc={rc}")
            try:
                yield
            finally:
                n = lib.axon_stop_nrt_profile(str(output_dir).encode())
                print(f"ntff profile: {n} file(s) -> {output_dir}",
                      file=sys.stderr)

        return _hook

    mod = types.ModuleType("antenv.axon_hooks")
    mod.set_axon_ntff_profile_hook = lambda h: holder.__setitem__("hook", h)
    mod.get_axon_ntff_profile_hook = lambda: holder["hook"]
    sys.modules["antenv.axon_hooks"] = mod
    holder["hook"] = _build()


# revision 21
# speedup vs baseline: 1.8839x; 1.0106x over previous
"""Causal self-attention (B=2, L=4096, D=768, H=12) on 8 Trainium2 cores.

Sharding: core c = (b, g) with b = c // 4, g = c % 4. Data-parallel over the
batch, tensor-parallel over heads (3 heads per core). Each core computes its
heads' attention over the full sequence plus its slice of the output
projection (row-parallel); the host sums the 4 partial projections per batch
element and adds b_out.

Device-side design (all matmul operands bf16, fp32 PSUM accumulation):
  - host supplies x[b]^T (768, L); q^T/k^T are produced in [dh, L] layout by
    the projection itself, v in natural [L, dh] layout — no on-device
    transposes anywhere.
  - scores are computed transposed, S^T[lk, lq], 512 lq per PSUM tile; exp
    runs on ScalarE with the 1/sqrt(dh) scale folded in and no
    max-subtraction (scores are ~N(0,1) here, exp is safe in fp32).
  - causal mask = multiply by a 0/1 tile, needed only on the 4 diagonal
    lk-tiles of each lq chunk.
  - PV matmul uses lhsT = [v | ones] so PSUM row 64 accumulates the softmax
    denominator for free; the reciprocal row is partition-broadcast on the
    (otherwise idle) GpSimd engine and applied by one DVE multiply.
  - the PE clock-gate (HAM) only sustains 2.4 GHz when the PE never idles,
    so the qkv projection of the NEXT lq chunk and the output projection of
    the PREVIOUS chunk are emitted as filler work inside the attention
    loop, and PV runs software-pipelined one lk-tile behind the exp.
"""

import os
import sys

sys.path.insert(0, "/opt/trn_rl_repo")

import numpy as np
import ml_dtypes

import concourse.bass as bass  # noqa: F401  (registers AP machinery)
import concourse.mybir as mybir
from concourse import bacc
import concourse.tile as tile
from concourse.bass_utils import run_bass_kernel_spmd

BF16 = ml_dtypes.bfloat16
F32 = mybir.dt.float32
BF = mybir.dt.bfloat16

D_MODEL = 768
N_HEADS = 12
D_HEAD = 64
B = 2
L_FULL = 4096
N_CORES = 8
TPG = 4  # head-groups (tensor-parallel degree per batch element)
HPC = N_HEADS // TPG  # 3 heads per core
DG = HPC * D_HEAD  # 192 feature dims per core
SCALE = 1.0 / np.sqrt(D_HEAD)

DM_CHUNKS = D_MODEL // 128  # 6


def build_nc(L=L_FULL):
    """Build the per-core Bass program (same program for all 8 cores)."""
    LC = L // 512  # lq chunks
    LT = L // 128  # lk / l tiles
    nc = bacc.Bacc("TRN2", target_bir_lowering=False, debug=False,
                   num_devices=N_CORES)

    xT_d = nc.dram_tensor("xT", [D_MODEL, L], BF, kind="ExternalInput").ap()
    wqk_d = nc.dram_tensor("wqkc", [D_MODEL, 512], BF, kind="ExternalInput").ap()
    bqk_d = nc.dram_tensor("bqkt", [128, 4], F32, kind="ExternalInput").ap()
    wv_d = nc.dram_tensor("wv", [D_MODEL, DG], BF, kind="ExternalInput").ap()
    wo2_d = nc.dram_tensor("wo2", [128, D_MODEL], BF, kind="ExternalInput").ap()
    wo3_d = nc.dram_tensor("wo3", [128, D_MODEL], BF, kind="ExternalInput").ap()
    mask_d = nc.dram_tensor("masks", [128, 4, 512], BF, kind="ExternalInput").ap()
    ROWW = max(L, (L // 128) * HPC * 65)
    rowz_d = nc.dram_tensor("rowz", [2, ROWW], BF, kind="ExternalInput").ap()
    out_d = nc.dram_tensor("out", [L, D_MODEL], F32, kind="ExternalOutput").ap()

    def bcast_row(src, nparts):
        return bass.AP(tensor=src.tensor, offset=src.offset,
                       ap=[[0, nparts]] + [list(src.ap[-1])])

    with tile.TileContext(nc) as tc:
        with tc.tile_pool(name="persist", bufs=1) as persist:
            xT_sb = persist.tile([128, DM_CHUNKS, L], BF)
            wqk_sb = persist.tile([128, DM_CHUNKS, 512], BF)
            wv_sb = persist.tile([128, DM_CHUNKS, DG], BF)
            bqk_sb = persist.tile([128, 4], F32)
            wo2_sb = persist.tile([128, D_MODEL], BF)
            wo3_sb = persist.tile([128, D_MODEL], BF)
            mask_sb = persist.tile([128, 4, 512], BF)
            # per-head q^T/k^T, zero-padded to K=128 (rows 64-127 stay zero:
            # a matmul that follows a K=64 matmul pays a ~100ns drain penalty)
            qT = [persist.tile([128, L], BF, name=f"qT{h}") for h in range(HPC)]
            kT = [persist.tile([128, L], BF, name=f"kT{h}") for h in range(HPC)]
            vones = persist.tile([128, LT, HPC * 65], BF)
            attnT01 = persist.tile([128, L], BF)
            attnT2 = persist.tile([128, L], BF)

            # zero-fill the K-padding rows and seed the [v|1] ones columns via
            # broadcast-DMA from constant DRAM rows (serial DVE memsets would
            # head-block the projection evacuations for ~25us)
            for h in range(HPC):
                nc.sync.dma_start(out=qT[h][64:128, :],
                                  in_=bcast_row(rowz_d[0:1, 0:L], 64))
                nc.sync.dma_start(out=kT[h][64:128, :],
                                  in_=bcast_row(rowz_d[0:1, 0:L], 64))
            nc.sync.dma_start(out=attnT2[64:128, :],
                              in_=bcast_row(rowz_d[0:1, 0:L], 64))
            nc.sync.dma_start(
                out=vones.rearrange("p a b -> p (a b)"),
                in_=bcast_row(rowz_d[1:2, 0:LT * HPC * 65], 128))
            for cdm in range(DM_CHUNKS):
                nc.sync.dma_start(out=wqk_sb[:, cdm, :],
                                  in_=wqk_d[cdm * 128:(cdm + 1) * 128, :])
            # x^T in lq-ordered strips so chunk 0's projections start early
            XSTRIP = min(1024, L)
            for cdm in range(DM_CHUNKS):
                nc.sync.dma_start(
                    out=xT_sb[:, cdm, 0:XSTRIP],
                    in_=xT_d[cdm * 128:(cdm + 1) * 128, 0:XSTRIP])
            for cdm in range(DM_CHUNKS):
                nc.sync.dma_start(out=wv_sb[:, cdm, :],
                                  in_=wv_d[cdm * 128:(cdm + 1) * 128, :])
            nc.sync.dma_start(out=bqk_sb, in_=bqk_d)
            nc.sync.dma_start(out=wo2_sb, in_=wo2_d)
            nc.sync.dma_start(out=wo3_sb, in_=wo3_d)
            nc.sync.dma_start(out=mask_sb, in_=mask_d)
            for ls in range(1, L // XSTRIP):
                for cdm in range(DM_CHUNKS):
                    nc.sync.dma_start(
                        out=xT_sb[:, cdm, ls * XSTRIP:(ls + 1) * XSTRIP],
                        in_=xT_d[cdm * 128:(cdm + 1) * 128,
                                 ls * XSTRIP:(ls + 1) * XSTRIP])

            # wqkc column chunks: 0=[q0|q1] 1=[k0|k1] 2=[q2|junk] 3=[k2|junk]
            # chunk evacuates into per-head tiles: psum rows 0-63 -> head a
            # rows 0-63, psum rows 64-127 -> head b rows 0-63 (shifted copy)
            qk_dest = [
                (qT[0], qT[1]),
                (kT[0], kT[1]),
                (qT[2], None),
                (kT[2], None),
            ]
            with (
                tc.tile_pool(name="p1psum", bufs=2, space="PSUM") as p1p,
                tc.tile_pool(name="stpsum", bufs=2, space="PSUM") as stp,
                tc.tile_pool(name="pvpsum", bufs=3, space="PSUM") as pvp,
                tc.tile_pool(name="popsum", bufs=1, space="PSUM") as pop,
                tc.tile_pool(name="ptpool", bufs=8) as ptp,
                tc.tile_pool(name="rpool", bufs=2) as rp,
                tc.tile_pool(name="outpool", bufs=3) as outp,
            ):
                def emit_qk(fc, lc):
                    dest_a, dest_b = qk_dest[fc]
                    ps = p1p.tile([128, 512], F32, tag="p1",
                                  name=f"psqk{fc}_{lc}")
                    for cdm in range(DM_CHUNKS):
                        nc.tensor.matmul(
                            ps,
                            wqk_sb[:, cdm, fc * 128:(fc + 1) * 128],
                            xT_sb[:, cdm, lc * 512:(lc + 1) * 512],
                            start=(cdm == 0), stop=(cdm == DM_CHUNKS - 1),
                        )
                    # evacuate with fused per-partition (=feature) bias add
                    nc.vector.tensor_scalar_add(
                        dest_a[0:64, lc * 512:(lc + 1) * 512],
                        ps[0:64, :],
                        bqk_sb[0:64, fc:fc + 1],
                    )
                    if dest_b is not None:
                        nc.vector.tensor_scalar_add(
                            dest_b[0:64, lc * 512:(lc + 1) * 512],
                            ps[64:128, :],
                            bqk_sb[64:128, fc:fc + 1],
                        )

                def emit_v(lt):
                    ps = p1p.tile([128, DG], F32, tag="p1", name=f"psv{lt}")
                    for cdm in range(DM_CHUNKS):
                        nc.tensor.matmul(
                            ps,
                            xT_sb[:, cdm, lt * 128:(lt + 1) * 128],
                            wv_sb[:, cdm, :],
                            start=(cdm == 0), stop=(cdm == DM_CHUNKS - 1),
                        )
                    nc.vector.tensor_copy(
                        vones[:, lt, 0:HPC * 65]
                        .rearrange("p (h c) -> p h c", h=HPC)[:, :, 0:64],
                        ps.rearrange("p (h c) -> p h c", h=HPC),
                    )

                def emit_proj(lt):
                    osb = outp.tile([128, D_MODEL], F32, tag="osb",
                                    name=f"osb{lt}")
                    for nh in range(2):
                        po = pop.tile([128, 384], F32, tag="po",
                                      name=f"po{lt}_{nh}")
                        nc.tensor.matmul(
                            po,
                            attnT01[:, lt * 128:(lt + 1) * 128],
                            wo2_sb[:, nh * 384:(nh + 1) * 384],
                            start=True, stop=False,
                        )
                        nc.tensor.matmul(
                            po,
                            attnT2[:, lt * 128:(lt + 1) * 128],
                            wo3_sb[:, nh * 384:(nh + 1) * 384],
                            start=False, stop=True,
                        )
                        nc.vector.tensor_copy(osb[:, nh * 384:(nh + 1) * 384], po)
                    nc.sync.dma_start(out=out_d[lt * 128:(lt + 1) * 128, :],
                                      in_=osb)

                def qkv_fillers(lc):
                    fs = [lambda fc=fc: emit_qk(fc, lc) for fc in range(4)]
                    fs += [lambda lt=lt: emit_v(lt)
                           for lt in range(4 * lc, 4 * lc + 4)]
                    return fs

                for f in qkv_fillers(0):
                    f()

                # (attnT tile, destination row base) per head
                norm_dest = [(attnT01, 0), (attnT01, 64), (attnT2, 0)]
                for c in range(LC):
                    nt = 4 * (c + 1)
                    fillers = qkv_fillers(c + 1) if c + 1 < LC else []
                    if c >= 1:
                        fillers += [lambda lt=lt: emit_proj(lt)
                                    for lt in range(4 * (c - 1), 4 * c)]
                    pv_acc = [pvp.tile([65, 512], F32, tag="pvacc",
                                       name=f"pvacc_c{c}h{h}")
                              for h in range(HPC)]
                    prev = []
                    fi = 0
                    for t in range(nt):
                        j = t - 4 * c  # >= 0 on diagonal tiles
                        col0 = 128 * j if j >= 0 else 0
                        cur = []
                        for h in range(HPC):
                            st = stp.tile([128, 512], F32, tag="st",
                                          name=f"st_c{c}t{t}h{h}")
                            nc.tensor.matmul(
                                st[:, col0:],
                                kT[h][:, t * 128:(t + 1) * 128],
                                qT[h][:, c * 512 + col0:(c + 1) * 512],
                            )
                            pt = ptp.tile([128, 512], BF, tag="pt",
                                          name=f"pt_c{c}t{t}h{h}")
                            nc.scalar.activation(
                                pt[:, col0:], st[:, col0:],
                                mybir.ActivationFunctionType.Exp,
                                scale=float(SCALE),
                            )
                            if j >= 0:
                                nc.vector.tensor_mul(
                                    pt[:, col0:], pt[:, col0:],
                                    mask_sb[:, j, col0:],
                                )
                            cur.append((h, pt, col0, t))
                        # PE filler work, spread across the lk-tile loop
                        want = (t + 1) * len(fillers) // nt
                        while fi < want:
                            fillers[fi]()
                            fi += 1
                        # software-pipelined PV: one lk-tile behind
                        for (h, pt0, c0, t0) in prev:
                            nc.tensor.matmul(
                                pv_acc[h][:, c0:],
                                vones[:, t0, h * 65:(h + 1) * 65],
                                pt0[:, c0:],
                                start=(t0 == 0), stop=False,
                            )
                        prev = cur
                    for (h, pt0, c0, t0) in prev:
                        nc.tensor.matmul(
                            pv_acc[h][:, c0:],
                            vones[:, t0, h * 65:(h + 1) * 65],
                            pt0[:, c0:],
                            start=(t0 == 0), stop=True,
                        )
                    prev = []
                    for h in range(HPC):
                        dn = rp.tile([1, 512], F32, tag="dn",
                                     name=f"dn_c{c}h{h}")
                        # partition-shifting copy (psum row 64 -> sbuf row 0);
                        # partition_broadcast only honors a partition-0 source
                        nc.vector.tensor_copy(dn[0:1, :],
                                              pv_acc[h][64:65, :])
                        dnb = rp.tile([64, 512], F32, tag="dnb",
                                      name=f"dnb_c{c}h{h}")
                        nc.gpsimd.partition_broadcast(dnb, dn[0:1, :])
                        rbs = rp.tile([64, 512], F32, tag="rbs",
                                      name=f"rbs_c{c}h{h}")
                        nc.vector.reciprocal_approx_fast(out=rbs, in_=dnb)
                        dt_, r0 = norm_dest[h]
                        nc.vector.tensor_mul(
                            dt_[r0:r0 + 64, c * 512:(c + 1) * 512],
                            pv_acc[h][0:64, :], rbs,
                        )
                for lt in range(4 * (LC - 1), LT):
                    emit_proj(lt)

    nc.compile()
    return nc


def make_in_maps(x, w_qkv, b_qkv, w_out, L=L_FULL):
    """Host-side sharding: build the 8 per-core input dicts."""
    # causal mask tiles for diagonal blocks: m[p, j, f] = (128 j + p) <= f
    p = np.arange(128)[:, None, None]
    jj = np.arange(4)[None, :, None]
    f = np.arange(512)[None, None, :]
    masks = ((128 * jj + p) <= f).astype(BF16)
    roww = max(L, (L // 128) * HPC * 65)
    rowz = np.zeros((2, roww), BF16)
    rowz[1] = 1.0

    xT = [np.ascontiguousarray(x[b].T.astype(BF16)) for b in range(B)]
    in_maps = []
    for c in range(N_CORES):
        b, g = divmod(c, TPG)
        h0 = g * HPC  # first global head of this group

        def qcol(h):
            return slice((h0 + h) * D_HEAD, (h0 + h + 1) * D_HEAD)

        def kcol(h):
            return slice(768 + (h0 + h) * D_HEAD, 768 + (h0 + h + 1) * D_HEAD)

        wqkc = np.zeros((D_MODEL, 512), np.float32)
        bqkc = np.zeros((512,), np.float32)
        # chunk0 [q0|q1], chunk1 [k0|k1], chunk2 [q2|-], chunk3 [k2|-]
        for h in range(2):
            wqkc[:, h * 64:(h + 1) * 64] = w_qkv[:, qcol(h)]
            wqkc[:, 128 + h * 64:128 + (h + 1) * 64] = w_qkv[:, kcol(h)]
            bqkc[h * 64:(h + 1) * 64] = b_qkv[qcol(h)]
            bqkc[128 + h * 64:128 + (h + 1) * 64] = b_qkv[kcol(h)]
        wqkc[:, 256:320] = w_qkv[:, qcol(2)]
        bqkc[256:320] = b_qkv[qcol(2)]
        wqkc[:, 384:448] = w_qkv[:, kcol(2)]
        bqkc[384:448] = b_qkv[kcol(2)]

        wv = w_qkv[:, 1536 + h0 * 64:1536 + (h0 + HPC) * 64]
        wo = w_out[h0 * 64:(h0 + HPC) * 64, :]
        wo3 = np.zeros((128, D_MODEL), np.float32)
        wo3[0:64] = wo[128:192]

        in_maps.append({
            "xT": xT[b][:, :L],
            "wqkc": wqkc.astype(BF16),
            "bqkt": np.ascontiguousarray(bqkc.reshape(4, 128).T),
            "wv": np.ascontiguousarray(wv).astype(BF16),
            "wo2": np.ascontiguousarray(wo[0:128]).astype(BF16),
            "wo3": wo3.astype(BF16),
            "masks": masks,
            "rowz": rowz,
        })
    return in_maps


_NC_CACHE = {}


def _get_nc(L=L_FULL):
    if L not in _NC_CACHE:
        _NC_CACHE[L] = build_nc(L)
    return _NC_CACHE[L]


def run(x, w_qkv, b_qkv, w_out, b_out, L=L_FULL, trace=False):
    nc = _get_nc(L)
    in_maps = make_in_maps(np.asarray(x), np.asarray(w_qkv),
                           np.asarray(b_qkv), np.asarray(w_out), L=L)
    if trace:
        install_ntff()
    res = run_bass_kernel_spmd(nc, in_maps, core_ids=list(range(N_CORES)),
                               trace=trace)
    partials = np.stack([res.results[c]["out"] for c in range(N_CORES)])
    out = partials.reshape(B, TPG, L, D_MODEL).sum(axis=1)
    # the V bias commutes through the attention average (weights sum to 1),
    # so it collapses to a constant row applied after the projection
    bias = np.asarray(b_qkv, np.float32)[1536:] @ np.asarray(w_out, np.float32)
    out = out + (bias + np.asarray(b_out, np.float32))[None, None, :]
    return out.astype(np.float32), res


def kernel(x, w_qkv, b_qkv, w_out, b_out):
    out, _ = run(x, w_qkv, b_qkv, w_out, b_out, L=L_FULL, trace=False)
    return out


# ---- optional NTFF profiling hook (axon images lack antenv.axon_hooks) ----
def install_ntff(so_path="/opt/axon/libaxon_pjrt.so"):
    import contextlib
    import ctypes
    import types

    if "antenv.axon_hooks" in sys.modules:
        return
    holder = {"hook": None}

    def _build():
        if not os.path.exists(so_path):
            return None
        lib = ctypes.CDLL(so_path)
        if not hasattr(lib, "axon_start_nrt_profile"):
            return None
        lib.axon_start_nrt_profile.argtypes = [ctypes.POINTER(ctypes.c_int64),
                                               ctypes.c_size_t]
        lib.axon_start_nrt_profile.restype = ctypes.c_int64
        lib.axon_stop_nrt_profile.argtypes = [ctypes.c_char_p]
        lib.axon_stop_nrt_profile.restype = ctypes.c_int64

        @contextlib.contextmanager
        def _hook(output_dir, device_ids):
            import jax
            jax.devices()
            if device_ids:
                ids = (ctypes.c_int64 * len(device_ids))(*device_ids)
                rc = lib.axon_start_nrt_profile(ids, len(device_ids))
            else:
                rc = lib.axon_start_nrt_profile(None, 0)
            if rc != 0:
                raise RuntimeError(f"axon_start_nrt_profile rc={rc}")
            try:
                yield
            finally:
                n = lib.axon_stop_nrt_profile(str(output_dir).encode())
                print(f"ntff profile: {n} file(s) -> {output_dir}",
                      file=sys.stderr)

        return _hook

    mod = types.ModuleType("antenv.axon_hooks")
    mod.set_axon_ntff_profile_hook = lambda h: holder.__setitem__("hook", h)
    mod.get_axon_ntff_profile_hook = lambda: holder["hook"]
    sys.modules["antenv.axon_hooks"] = mod
    holder["hook"] = _build()


# revision 25
# speedup vs baseline: 1.9919x; 1.0573x over previous
"""Causal self-attention (B=2, L=4096, D=768, H=12) on 8 Trainium2 cores.

Sharding: core c = (b, g) with b = c // 4, g = c % 4. Data-parallel over the
batch, tensor-parallel over heads (3 heads per core). Each core computes its
heads' attention over the full sequence plus its slice of the output
projection (row-parallel); the host sums the 4 partial projections per batch
element and adds b_out.

Device-side design (all matmul operands bf16, fp32 PSUM accumulation):
  - host supplies x[b]^T (768, L); q^T/k^T are produced in [dh, L] layout by
    the projection itself, v in natural [L, dh] layout — no on-device
    transposes anywhere.
  - scores are computed transposed, S^T[lk, lq], 512 lq per PSUM tile; exp
    runs on ScalarE with the 1/sqrt(dh) scale folded in and no
    max-subtraction (scores are ~N(0,1) here, exp is safe in fp32).
  - causal mask = multiply by a 0/1 tile, needed only on the 4 diagonal
    lk-tiles of each lq chunk.
  - PV matmul uses lhsT = [v | ones] so PSUM row 64 accumulates the softmax
    denominator for free; the reciprocal row is partition-broadcast on the
    (otherwise idle) GpSimd engine and applied by one DVE multiply.
  - the PE clock-gate (HAM) only sustains 2.4 GHz when the PE never idles,
    so the qkv projection of the NEXT lq chunk and the output projection of
    the PREVIOUS chunk are emitted as filler work inside the attention
    loop, and PV runs software-pipelined one lk-tile behind the exp.
"""

import os
import sys

sys.path.insert(0, "/opt/trn_rl_repo")

import numpy as np
import ml_dtypes

import concourse.bass as bass  # noqa: F401  (registers AP machinery)
import concourse.mybir as mybir
from concourse import bacc
import concourse.tile as tile
from concourse.bass_utils import run_bass_kernel_spmd

BF16 = ml_dtypes.bfloat16
F32 = mybir.dt.float32
BF = mybir.dt.bfloat16

D_MODEL = 768
N_HEADS = 12
D_HEAD = 64
B = 2
L_FULL = 4096
N_CORES = 8
TPG = 4  # head-groups (tensor-parallel degree per batch element)
HPC = N_HEADS // TPG  # 3 heads per core
DG = HPC * D_HEAD  # 192 feature dims per core
SCALE = 1.0 / np.sqrt(D_HEAD)

DM_CHUNKS = D_MODEL // 128  # 6


def build_nc(L=L_FULL):
    """Build the per-core Bass program (same program for all 8 cores)."""
    LC = L // 512  # lq chunks
    LT = L // 128  # lk / l tiles
    nc = bacc.Bacc("TRN2", target_bir_lowering=False, debug=False,
                   num_devices=N_CORES)

    xT_d = nc.dram_tensor("xT", [D_MODEL, L], BF, kind="ExternalInput").ap()
    wqk_d = nc.dram_tensor("wqkc", [D_MODEL, 512], BF, kind="ExternalInput").ap()
    bqk_d = nc.dram_tensor("bqkt", [128, 4], F32, kind="ExternalInput").ap()
    wv_d = nc.dram_tensor("wv", [D_MODEL, DG], BF, kind="ExternalInput").ap()
    wo2_d = nc.dram_tensor("wo2", [128, D_MODEL], BF, kind="ExternalInput").ap()
    wo3_d = nc.dram_tensor("wo3", [128, D_MODEL], BF, kind="ExternalInput").ap()
    mask_d = nc.dram_tensor("masks", [128, 4, 512], BF, kind="ExternalInput").ap()
    out_d = nc.dram_tensor("out", [L, D_MODEL], F32, kind="ExternalOutput").ap()

    with tile.TileContext(nc) as tc:
        with tc.tile_pool(name="persist", bufs=1) as persist:
            xT_sb = persist.tile([128, DM_CHUNKS, L], BF)
            wqk_sb = persist.tile([128, DM_CHUNKS, 512], BF)
            wv_sb = persist.tile([128, DM_CHUNKS, DG], BF)
            bqk_sb = persist.tile([128, 4], F32)
            wo2_sb = persist.tile([128, D_MODEL], BF)
            wo3_sb = persist.tile([128, D_MODEL], BF)
            mask_sb = persist.tile([128, 4, 512], BF)
            # per-head q^T/k^T, zero-padded to K=128 (rows 64-127 stay zero:
            # a matmul that follows a K=64 matmul pays a ~100ns drain penalty)
            qT = [persist.tile([128, L], BF, name=f"qT{h}") for h in range(HPC)]
            kT = [persist.tile([128, L], BF, name=f"kT{h}") for h in range(HPC)]
            vones = persist.tile([128, LT, HPC * 65], BF)
            attnT01 = persist.tile([128, L], BF)
            attnT2 = persist.tile([128, L], BF)

            # zero-fill the K-padding rows: first-needed on DVE (it is idle at
            # start and must not head-block the projection evacuations), the
            # rest on the otherwise-idle GpSimd, ordered by first use
            nc.vector.memset(kT[0][64:128, :], 0.0)
            nc.vector.memset(qT[0][64:128, :], 0.0)
            nc.vector.memset(vones, 1.0)
            nc.gpsimd.memset(kT[1][64:128, :], 0.0)
            nc.gpsimd.memset(qT[1][64:128, :], 0.0)
            nc.gpsimd.memset(kT[2][64:128, :], 0.0)
            nc.gpsimd.memset(qT[2][64:128, :], 0.0)
            nc.gpsimd.memset(attnT2[64:128, :], 0.0)
            for cdm in range(DM_CHUNKS):
                nc.sync.dma_start(out=wqk_sb[:, cdm, :],
                                  in_=wqk_d[cdm * 128:(cdm + 1) * 128, :])
            # x^T in lq-ordered strips so chunk 0's projections start early
            XSTRIP = min(1024, L)
            for cdm in range(DM_CHUNKS):
                nc.sync.dma_start(
                    out=xT_sb[:, cdm, 0:XSTRIP],
                    in_=xT_d[cdm * 128:(cdm + 1) * 128, 0:XSTRIP])
            for cdm in range(DM_CHUNKS):
                nc.sync.dma_start(out=wv_sb[:, cdm, :],
                                  in_=wv_d[cdm * 128:(cdm + 1) * 128, :])
            nc.sync.dma_start(out=bqk_sb, in_=bqk_d)
            nc.sync.dma_start(out=wo2_sb, in_=wo2_d)
            nc.sync.dma_start(out=wo3_sb, in_=wo3_d)
            nc.sync.dma_start(out=mask_sb, in_=mask_d)
            for ls in range(1, L // XSTRIP):
                for cdm in range(DM_CHUNKS):
                    nc.sync.dma_start(
                        out=xT_sb[:, cdm, ls * XSTRIP:(ls + 1) * XSTRIP],
                        in_=xT_d[cdm * 128:(cdm + 1) * 128,
                                 ls * XSTRIP:(ls + 1) * XSTRIP])

            # wqkc column chunks: 0=[q0|q1] 1=[k0|k1] 2=[q2|junk] 3=[k2|junk]
            # chunk evacuates into per-head tiles: psum rows 0-63 -> head a
            # rows 0-63, psum rows 64-127 -> head b rows 0-63 (shifted copy)
            qk_dest = [
                (qT[0], qT[1]),
                (kT[0], kT[1]),
                (qT[2], None),
                (kT[2], None),
            ]
            with (
                tc.tile_pool(name="p1psum", bufs=2, space="PSUM") as p1p,
                tc.tile_pool(name="stpsum", bufs=2, space="PSUM") as stp,
                tc.tile_pool(name="pvpsum", bufs=3, space="PSUM") as pvp,
                tc.tile_pool(name="popsum", bufs=1, space="PSUM") as pop,
                tc.tile_pool(name="ptpool", bufs=8) as ptp,
                tc.tile_pool(name="rpool", bufs=2) as rp,
                tc.tile_pool(name="outpool", bufs=3) as outp,
            ):
                def emit_qk(fc, lc):
                    dest_a, dest_b = qk_dest[fc]
                    ps = p1p.tile([128, 512], F32, tag="p1",
                                  name=f"psqk{fc}_{lc}")
                    for cdm in range(DM_CHUNKS):
                        nc.tensor.matmul(
                            ps,
                            wqk_sb[:, cdm, fc * 128:(fc + 1) * 128],
                            xT_sb[:, cdm, lc * 512:(lc + 1) * 512],
                            start=(cdm == 0), stop=(cdm == DM_CHUNKS - 1),
                        )
                    # evacuate with fused per-partition (=feature) bias add
                    nc.vector.tensor_scalar_add(
                        dest_a[0:64, lc * 512:(lc + 1) * 512],
                        ps[0:64, :],
                        bqk_sb[0:64, fc:fc + 1],
                    )
                    if dest_b is not None:
                        nc.vector.tensor_scalar_add(
                            dest_b[0:64, lc * 512:(lc + 1) * 512],
                            ps[64:128, :],
                            bqk_sb[64:128, fc:fc + 1],
                        )

                def emit_v(lt):
                    ps = p1p.tile([128, DG], F32, tag="p1", name=f"psv{lt}")
                    for cdm in range(DM_CHUNKS):
                        nc.tensor.matmul(
                            ps,
                            xT_sb[:, cdm, lt * 128:(lt + 1) * 128],
                            wv_sb[:, cdm, :],
                            start=(cdm == 0), stop=(cdm == DM_CHUNKS - 1),
                        )
                    nc.vector.tensor_copy(
                        vones[:, lt, 0:HPC * 65]
                        .rearrange("p (h c) -> p h c", h=HPC)[:, :, 0:64],
                        ps.rearrange("p (h c) -> p h c", h=HPC),
                    )

                def emit_proj(lt):
                    osb = outp.tile([128, D_MODEL], F32, tag="osb",
                                    name=f"osb{lt}")
                    for nh in range(2):
                        po = pop.tile([128, 384], F32, tag="po",
                                      name=f"po{lt}_{nh}")
                        nc.tensor.matmul(
                            po,
                            attnT01[:, lt * 128:(lt + 1) * 128],
                            wo2_sb[:, nh * 384:(nh + 1) * 384],
                            start=True, stop=False,
                        )
                        nc.tensor.matmul(
                            po,
                            attnT2[:, lt * 128:(lt + 1) * 128],
                            wo3_sb[:, nh * 384:(nh + 1) * 384],
                            start=False, stop=True,
                        )
                        nc.vector.tensor_copy(osb[:, nh * 384:(nh + 1) * 384], po)
                    nc.sync.dma_start(out=out_d[lt * 128:(lt + 1) * 128, :],
                                      in_=osb)

                def qkv_fillers(lc):
                    fs = [lambda fc=fc: emit_qk(fc, lc) for fc in range(4)]
                    fs += [lambda lt=lt: emit_v(lt)
                           for lt in range(4 * lc, 4 * lc + 4)]
                    return fs

                for f in qkv_fillers(0):
                    f()

                # (attnT tile, destination row base) per head
                norm_dest = [(attnT01, 0), (attnT01, 64), (attnT2, 0)]
                for c in range(LC):
                    nt = 4 * (c + 1)
                    fillers = qkv_fillers(c + 1) if c + 1 < LC else []
                    if c >= 1:
                        fillers += [lambda lt=lt: emit_proj(lt)
                                    for lt in range(4 * (c - 1), 4 * c)]
                    pv_acc = [pvp.tile([65, 512], F32, tag="pvacc",
                                       name=f"pvacc_c{c}h{h}")
                              for h in range(HPC)]
                    prev = []
                    fi = 0
                    for t in range(nt):
                        j = t - 4 * c  # >= 0 on diagonal tiles
                        col0 = 128 * j if j >= 0 else 0
                        cur = []
                        for h in range(HPC):
                            st = stp.tile([128, 512], F32, tag="st",
                                          name=f"st_c{c}t{t}h{h}")
                            nc.tensor.matmul(
                                st[:, col0:],
                                kT[h][:, t * 128:(t + 1) * 128],
                                qT[h][:, c * 512 + col0:(c + 1) * 512],
                            )
                            pt = ptp.tile([128, 512], BF, tag="pt",
                                          name=f"pt_c{c}t{t}h{h}")
                            nc.scalar.activation(
                                pt[:, col0:], st[:, col0:],
                                mybir.ActivationFunctionType.Exp,
                                scale=float(SCALE),
                            )
                            if j >= 0:
                                nc.vector.tensor_mul(
                                    pt[:, col0:], pt[:, col0:],
                                    mask_sb[:, j, col0:],
                                )
                            cur.append((h, pt, col0, t))
                        # PE filler work, spread across the lk-tile loop
                        want = (t + 1) * len(fillers) // nt
                        while fi < want:
                            fillers[fi]()
                            fi += 1
                        # software-pipelined PV: one lk-tile behind
                        for (h, pt0, c0, t0) in prev:
                            nc.tensor.matmul(
                                pv_acc[h][:, c0:],
                                vones[:, t0, h * 65:(h + 1) * 65],
                                pt0[:, c0:],
                                start=(t0 == 0), stop=False,
                            )
                        prev = cur
                    for (h, pt0, c0, t0) in prev:
                        nc.tensor.matmul(
                            pv_acc[h][:, c0:],
                            vones[:, t0, h * 65:(h + 1) * 65],
                            pt0[:, c0:],
                            start=(t0 == 0), stop=True,
                        )
                    prev = []
                    for h in range(HPC):
                        dn = rp.tile([1, 512], F32, tag="dn",
                                     name=f"dn_c{c}h{h}")
                        # partition-shifting copy (psum row 64 -> sbuf row 0);
                        # partition_broadcast only honors a partition-0 source
                        nc.vector.tensor_copy(dn[0:1, :],
                                              pv_acc[h][64:65, :])
                        dnb = rp.tile([64, 512], F32, tag="dnb",
                                      name=f"dnb_c{c}h{h}")
                        nc.gpsimd.partition_broadcast(dnb, dn[0:1, :])
                        rbs = rp.tile([64, 512], F32, tag="rbs",
                                      name=f"rbs_c{c}h{h}")
                        nc.vector.reciprocal_approx_fast(out=rbs, in_=dnb)
                        dt_, r0 = norm_dest[h]
                        nc.vector.tensor_mul(
                            dt_[r0:r0 + 64, c * 512:(c + 1) * 512],
                            pv_acc[h][0:64, :], rbs,
                        )
                for lt in range(4 * (LC - 1), LT):
                    emit_proj(lt)

    nc.compile()
    return nc


def make_in_maps(x, w_qkv, b_qkv, w_out, L=L_FULL):
    """Host-side sharding: build the 8 per-core input dicts."""
    # causal mask tiles for diagonal blocks: m[p, j, f] = (128 j + p) <= f
    p = np.arange(128)[:, None, None]
    jj = np.arange(4)[None, :, None]
    f = np.arange(512)[None, None, :]
    masks = ((128 * jj + p) <= f).astype(BF16)

    xT = [np.ascontiguousarray(x[b].T.astype(BF16)) for b in range(B)]
    in_maps = []
    for c in range(N_CORES):
        b, g = divmod(c, TPG)
        h0 = g * HPC  # first global head of this group

        def qcol(h):
            return slice((h0 + h) * D_HEAD, (h0 + h + 1) * D_HEAD)

        def kcol(h):
            return slice(768 + (h0 + h) * D_HEAD, 768 + (h0 + h + 1) * D_HEAD)

        wqkc = np.zeros((D_MODEL, 512), np.float32)
        bqkc = np.zeros((512,), np.float32)
        # chunk0 [q0|q1], chunk1 [k0|k1], chunk2 [q2|-], chunk3 [k2|-]
        for h in range(2):
            wqkc[:, h * 64:(h + 1) * 64] = w_qkv[:, qcol(h)]
            wqkc[:, 128 + h * 64:128 + (h + 1) * 64] = w_qkv[:, kcol(h)]
            bqkc[h * 64:(h + 1) * 64] = b_qkv[qcol(h)]
            bqkc[128 + h * 64:128 + (h + 1) * 64] = b_qkv[kcol(h)]
        wqkc[:, 256:320] = w_qkv[:, qcol(2)]
        bqkc[256:320] = b_qkv[qcol(2)]
        wqkc[:, 384:448] = w_qkv[:, kcol(2)]
        bqkc[384:448] = b_qkv[kcol(2)]

        wv = w_qkv[:, 1536 + h0 * 64:1536 + (h0 + HPC) * 64]
        wo = w_out[h0 * 64:(h0 + HPC) * 64, :]
        wo3 = np.zeros((128, D_MODEL), np.float32)
        wo3[0:64] = wo[128:192]

        in_maps.append({
            "xT": xT[b][:, :L],
            "wqkc": wqkc.astype(BF16),
            "bqkt": np.ascontiguousarray(bqkc.reshape(4, 128).T),
            "wv": np.ascontiguousarray(wv).astype(BF16),
            "wo2": np.ascontiguousarray(wo[0:128]).astype(BF16),
            "wo3": wo3.astype(BF16),
            "masks": masks,
        })
    return in_maps


_NC_CACHE = {}


def _get_nc(L=L_FULL):
    if L not in _NC_CACHE:
        _NC_CACHE[L] = build_nc(L)
    return _NC_CACHE[L]


def run(x, w_qkv, b_qkv, w_out, b_out, L=L_FULL, trace=False):
    nc = _get_nc(L)
    in_maps = make_in_maps(np.asarray(x), np.asarray(w_qkv),
                           np.asarray(b_qkv), np.asarray(w_out), L=L)
    if trace:
        install_ntff()
    res = run_bass_kernel_spmd(nc, in_maps, core_ids=list(range(N_CORES)),
                               trace=trace)
    partials = np.stack([res.results[c]["out"] for c in range(N_CORES)])
    out = partials.reshape(B, TPG, L, D_MODEL).sum(axis=1)
    # the V bias commutes through the attention average (weights sum to 1),
    # so it collapses to a constant row applied after the projection
    bias = np.asarray(b_qkv, np.float32)[1536:] @ np.asarray(w_out, np.float32)
    out = out + (bias + np.asarray(b_out, np.float32))[None, None, :]
    return out.astype(np.float32), res


def kernel(x, w_qkv, b_qkv, w_out, b_out):
    out, _ = run(x, w_qkv, b_qkv, w_out, b_out, L=L_FULL, trace=False)
    return out


# ---- optional NTFF profiling hook (axon images lack antenv.axon_hooks) ----
def install_ntff(so_path="/opt/axon/libaxon_pjrt.so"):
    import contextlib
    import ctypes
    import types

    if "antenv.axon_hooks" in sys.modules:
        return
    holder = {"hook": None}

    def _build():
        if not os.path.exists(so_path):
            return None
        lib = ctypes.CDLL(so_path)
        if not hasattr(lib, "axon_start_nrt_profile"):
            return None
        lib.axon_start_nrt_profile.argtypes = [ctypes.POINTER(ctypes.c_int64),
                                               ctypes.c_size_t]
        lib.axon_start_nrt_profile.restype = ctypes.c_int64
        lib.axon_stop_nrt_profile.argtypes = [ctypes.c_char_p]
        lib.axon_stop_nrt_profile.restype = ctypes.c_int64

        @contextlib.contextmanager
        def _hook(output_dir, device_ids):
            import jax
            jax.devices()
            if device_ids:
                ids = (ctypes.c_int64 * len(device_ids))(*device_ids)
                rc = lib.axon_start_nrt_profile(ids, len(device_ids))
            else:
                rc = lib.axon_start_nrt_profile(None, 0)
            if rc != 0:
                raise RuntimeError(f"axon_start_nrt_profile rc={rc}")
            try:
                yield
            finally:
                n = lib.axon_stop_nrt_profile(str(output_dir).encode())
                print(f"ntff profile: {n} file(s) -> {output_dir}",
                      file=sys.stderr)

        return _hook

    mod = types.ModuleType("antenv.axon_hooks")
    mod.set_axon_ntff_profile_hook = lambda h: holder.__setitem__("hook", h)
    mod.get_axon_ntff_profile_hook = lambda: holder["hook"]
    sys.modules["antenv.axon_hooks"] = mod
    holder["hook"] = _build()


# revision 27
# speedup vs baseline: 2.0807x; 1.0446x over previous
"""Causal self-attention (B=2, L=4096, D=768, H=12) on 8 Trainium2 cores.

Sharding: core c = (b, g) with b = c // 4, g = c % 4. Data-parallel over the
batch, tensor-parallel over heads (3 heads per core). Each core computes its
heads' attention over the full sequence plus its slice of the output
projection (row-parallel); the host sums the 4 partial projections per batch
element and adds b_out.

Device-side design (all matmul operands bf16, fp32 PSUM accumulation):
  - host supplies x[b]^T (768, L); q^T/k^T are produced in [dh, L] layout by
    the projection itself, v in natural [L, dh] layout — no on-device
    transposes anywhere.
  - scores are computed transposed, S^T[lk, lq], 512 lq per PSUM tile; exp
    runs on ScalarE with the 1/sqrt(dh) scale folded in and no
    max-subtraction (scores are ~N(0,1) here, exp is safe in fp32).
  - causal mask = multiply by a 0/1 tile, needed only on the 4 diagonal
    lk-tiles of each lq chunk.
  - PV matmul uses lhsT = [v | ones] so PSUM row 64 accumulates the softmax
    denominator for free; the reciprocal row is partition-broadcast on the
    (otherwise idle) GpSimd engine and applied by one DVE multiply.
  - the PE clock-gate (HAM) only sustains 2.4 GHz when the PE never idles,
    so the qkv projection of the NEXT lq chunk and the output projection of
    the PREVIOUS chunk are emitted as filler work inside the attention
    loop, and PV runs software-pipelined one lk-tile behind the exp.
"""

import os
import sys

sys.path.insert(0, "/opt/trn_rl_repo")

import numpy as np
import ml_dtypes

import concourse.bass as bass  # noqa: F401  (registers AP machinery)
import concourse.mybir as mybir
from concourse import bacc
import concourse.tile as tile
from concourse.bass_utils import run_bass_kernel_spmd

BF16 = ml_dtypes.bfloat16
F32 = mybir.dt.float32
BF = mybir.dt.bfloat16

D_MODEL = 768
N_HEADS = 12
D_HEAD = 64
B = 2
L_FULL = 4096
N_CORES = 8
TPG = 4  # head-groups (tensor-parallel degree per batch element)
HPC = N_HEADS // TPG  # 3 heads per core
DG = HPC * D_HEAD  # 192 feature dims per core
SCALE = 1.0 / np.sqrt(D_HEAD)

DM_CHUNKS = D_MODEL // 128  # 6


def build_nc(L=L_FULL):
    """Build the per-core Bass program (same program for all 8 cores)."""
    LC = L // 512  # lq chunks
    LT = L // 128  # lk / l tiles
    nc = bacc.Bacc("TRN2", target_bir_lowering=False, debug=False,
                   num_devices=N_CORES)

    xT_d = nc.dram_tensor("xT", [D_MODEL, L], BF, kind="ExternalInput").ap()
    wqk_d = nc.dram_tensor("wqkc", [D_MODEL, 512], BF, kind="ExternalInput").ap()
    bqk_d = nc.dram_tensor("bqkt", [128, 4], F32, kind="ExternalInput").ap()
    wv_d = nc.dram_tensor("wv", [D_MODEL, DG], BF, kind="ExternalInput").ap()
    wo2_d = nc.dram_tensor("wo2", [128, D_MODEL], BF, kind="ExternalInput").ap()
    wo3_d = nc.dram_tensor("wo3", [128, D_MODEL], BF, kind="ExternalInput").ap()
    mask_d = nc.dram_tensor("masks", [128, 4, 512], BF, kind="ExternalInput").ap()
    out_d = nc.dram_tensor("out", [L, D_MODEL], F32, kind="ExternalOutput").ap()

    with tile.TileContext(nc) as tc:
        with tc.tile_pool(name="persist", bufs=1) as persist:
            xT_sb = persist.tile([128, DM_CHUNKS, L], BF)
            wqk_sb = persist.tile([128, DM_CHUNKS, 512], BF)
            wv_sb = persist.tile([128, DM_CHUNKS, DG], BF)
            bqk_sb = persist.tile([128, 4], F32)
            wo2_sb = persist.tile([128, D_MODEL], BF)
            wo3_sb = persist.tile([128, D_MODEL], BF)
            mask_sb = persist.tile([128, 4, 512], BF)
            # per-head q^T/k^T, zero-padded to K=128 (rows 64-127 stay zero:
            # a matmul that follows a K=64 matmul pays a ~100ns drain penalty)
            qT = [persist.tile([128, L], BF, name=f"qT{h}") for h in range(HPC)]
            kT = [persist.tile([128, L], BF, name=f"kT{h}") for h in range(HPC)]
            vones = persist.tile([128, LT, HPC * 65], BF)
            attnT01 = persist.tile([128, L], BF)
            attnT2 = persist.tile([128, L], BF)

            # zero-fill the K-padding rows: first-needed on DVE (it is idle at
            # start and must not head-block the projection evacuations), the
            # rest on the otherwise-idle GpSimd, ordered by first use
            nc.vector.memset(kT[0][64:128, :], 0.0)
            nc.vector.memset(qT[0][64:128, :], 0.0)
            nc.vector.memset(vones, 1.0)
            nc.gpsimd.memset(kT[1][64:128, :], 0.0)
            nc.gpsimd.memset(qT[1][64:128, :], 0.0)
            nc.gpsimd.memset(kT[2][64:128, :], 0.0)
            nc.gpsimd.memset(qT[2][64:128, :], 0.0)
            nc.gpsimd.memset(attnT2[64:128, :], 0.0)
            for cdm in range(DM_CHUNKS):
                nc.sync.dma_start(out=wqk_sb[:, cdm, :],
                                  in_=wqk_d[cdm * 128:(cdm + 1) * 128, :])
            # x^T in lq-ordered strips so chunk 0's projections start early
            XSTRIP = min(1024, L)
            for cdm in range(DM_CHUNKS):
                nc.sync.dma_start(
                    out=xT_sb[:, cdm, 0:XSTRIP],
                    in_=xT_d[cdm * 128:(cdm + 1) * 128, 0:XSTRIP])
            for cdm in range(DM_CHUNKS):
                nc.sync.dma_start(out=wv_sb[:, cdm, :],
                                  in_=wv_d[cdm * 128:(cdm + 1) * 128, :])
            nc.sync.dma_start(out=bqk_sb, in_=bqk_d)
            nc.sync.dma_start(out=wo2_sb, in_=wo2_d)
            nc.sync.dma_start(out=wo3_sb, in_=wo3_d)
            nc.sync.dma_start(out=mask_sb, in_=mask_d)
            for ls in range(1, L // XSTRIP):
                for cdm in range(DM_CHUNKS):
                    nc.sync.dma_start(
                        out=xT_sb[:, cdm, ls * XSTRIP:(ls + 1) * XSTRIP],
                        in_=xT_d[cdm * 128:(cdm + 1) * 128,
                                 ls * XSTRIP:(ls + 1) * XSTRIP])

            # wqkc column chunks: 0=[q0|q1] 1=[k0|k1] 2=[q2|junk] 3=[k2|junk]
            # chunk evacuates into per-head tiles: psum rows 0-63 -> head a
            # rows 0-63, psum rows 64-127 -> head b rows 0-63 (shifted copy)
            qk_dest = [
                (qT[0], qT[1]),
                (kT[0], kT[1]),
                (qT[2], None),
                (kT[2], None),
            ]
            with (
                tc.tile_pool(name="p1psum", bufs=2, space="PSUM") as p1p,
                tc.tile_pool(name="stpsum", bufs=3, space="PSUM") as stp,
                tc.tile_pool(name="pvpsum", bufs=3, space="PSUM") as pvp,
                tc.tile_pool(name="ptpool", bufs=8) as ptp,
                tc.tile_pool(name="rpool", bufs=2) as rp,
                tc.tile_pool(name="outpool", bufs=3) as outp,
            ):
                def emit_qk(fc, lc):
                    dest_a, dest_b = qk_dest[fc]
                    ps = p1p.tile([128, 512], F32, tag="p1",
                                  name=f"psqk{fc}_{lc}")
                    for cdm in range(DM_CHUNKS):
                        nc.tensor.matmul(
                            ps,
                            wqk_sb[:, cdm, fc * 128:(fc + 1) * 128],
                            xT_sb[:, cdm, lc * 512:(lc + 1) * 512],
                            start=(cdm == 0), stop=(cdm == DM_CHUNKS - 1),
                        )
                    # evacuate with fused per-partition (=feature) bias add
                    nc.vector.tensor_scalar_add(
                        dest_a[0:64, lc * 512:(lc + 1) * 512],
                        ps[0:64, :],
                        bqk_sb[0:64, fc:fc + 1],
                    )
                    if dest_b is not None:
                        nc.vector.tensor_scalar_add(
                            dest_b[0:64, lc * 512:(lc + 1) * 512],
                            ps[64:128, :],
                            bqk_sb[64:128, fc:fc + 1],
                        )

                def emit_v(lt):
                    ps = p1p.tile([128, DG], F32, tag="p1", name=f"psv{lt}")
                    for cdm in range(DM_CHUNKS):
                        nc.tensor.matmul(
                            ps,
                            xT_sb[:, cdm, lt * 128:(lt + 1) * 128],
                            wv_sb[:, cdm, :],
                            start=(cdm == 0), stop=(cdm == DM_CHUNKS - 1),
                        )
                    nc.vector.tensor_copy(
                        vones[:, lt, 0:HPC * 65]
                        .rearrange("p (h c) -> p h c", h=HPC)[:, :, 0:64],
                        ps.rearrange("p (h c) -> p h c", h=HPC),
                    )

                def emit_proj(lt):
                    osb = outp.tile([128, D_MODEL], F32, tag="osb",
                                    name=f"osb{lt}")
                    for nh in range(2):
                        po = p1p.tile([128, 384], F32, tag="p1",
                                      name=f"po{lt}_{nh}")
                        nc.tensor.matmul(
                            po,
                            attnT01[:, lt * 128:(lt + 1) * 128],
                            wo2_sb[:, nh * 384:(nh + 1) * 384],
                            start=True, stop=False,
                        )
                        nc.tensor.matmul(
                            po,
                            attnT2[:, lt * 128:(lt + 1) * 128],
                            wo3_sb[:, nh * 384:(nh + 1) * 384],
                            start=False, stop=True,
                        )
                        nc.vector.tensor_copy(osb[:, nh * 384:(nh + 1) * 384], po)
                    nc.sync.dma_start(out=out_d[lt * 128:(lt + 1) * 128, :],
                                      in_=osb)

                def qkv_fillers(lc):
                    fs = [lambda fc=fc: emit_qk(fc, lc) for fc in range(4)]
                    fs += [lambda lt=lt: emit_v(lt)
                           for lt in range(4 * lc, 4 * lc + 4)]
                    return fs

                for f in qkv_fillers(0):
                    f()

                # (attnT tile, destination row base) per head
                norm_dest = [(attnT01, 0), (attnT01, 64), (attnT2, 0)]
                for c in range(LC):
                    nt = 4 * (c + 1)
                    fillers = qkv_fillers(c + 1) if c + 1 < LC else []
                    if c >= 1:
                        fillers += [lambda lt=lt: emit_proj(lt)
                                    for lt in range(4 * (c - 1), 4 * c)]
                    pv_acc = [pvp.tile([65, 512], F32, tag="pvacc",
                                       name=f"pvacc_c{c}h{h}")
                              for h in range(HPC)]
                    prev = []
                    fi = 0
                    for t in range(nt):
                        j = t - 4 * c  # >= 0 on diagonal tiles
                        col0 = 128 * j if j >= 0 else 0
                        cur = []
                        for h in range(HPC):
                            st = stp.tile([128, 512], F32, tag="st",
                                          name=f"st_c{c}t{t}h{h}")
                            nc.tensor.matmul(
                                st[:, col0:],
                                kT[h][:, t * 128:(t + 1) * 128],
                                qT[h][:, c * 512 + col0:(c + 1) * 512],
                            )
                            pt = ptp.tile([128, 512], BF, tag="pt",
                                          name=f"pt_c{c}t{t}h{h}")
                            nc.scalar.activation(
                                pt[:, col0:], st[:, col0:],
                                mybir.ActivationFunctionType.Exp,
                                scale=float(SCALE),
                            )
                            if j >= 0:
                                nc.vector.tensor_mul(
                                    pt[:, col0:], pt[:, col0:],
                                    mask_sb[:, j, col0:],
                                )
                            cur.append((h, pt, col0, t))
                        # PE filler work, spread across the lk-tile loop
                        want = (t + 1) * len(fillers) // nt
                        while fi < want:
                            fillers[fi]()
                            fi += 1
                        # software-pipelined PV: one lk-tile behind
                        for (h, pt0, c0, t0) in prev:
                            nc.tensor.matmul(
                                pv_acc[h][:, c0:],
                                vones[:, t0, h * 65:(h + 1) * 65],
                                pt0[:, c0:],
                                start=(t0 == 0), stop=False,
                            )
                        prev = cur
                    for (h, pt0, c0, t0) in prev:
                        nc.tensor.matmul(
                            pv_acc[h][:, c0:],
                            vones[:, t0, h * 65:(h + 1) * 65],
                            pt0[:, c0:],
                            start=(t0 == 0), stop=True,
                        )
                    prev = []
                    for h in range(HPC):
                        dn = rp.tile([1, 512], F32, tag="dn",
                                     name=f"dn_c{c}h{h}")
                        # partition-shifting copy (psum row 64 -> sbuf row 0);
                        # partition_broadcast only honors a partition-0 source
                        nc.vector.tensor_copy(dn[0:1, :],
                                              pv_acc[h][64:65, :])
                        dnb = rp.tile([64, 512], F32, tag="dnb",
                                      name=f"dnb_c{c}h{h}")
                        nc.gpsimd.partition_broadcast(dnb, dn[0:1, :])
                        rbs = rp.tile([64, 512], F32, tag="rbs",
                                      name=f"rbs_c{c}h{h}")
                        nc.vector.reciprocal_approx_fast(out=rbs, in_=dnb)
                        dt_, r0 = norm_dest[h]
                        nc.vector.tensor_mul(
                            dt_[r0:r0 + 64, c * 512:(c + 1) * 512],
                            pv_acc[h][0:64, :], rbs,
                        )
                for lt in range(4 * (LC - 1), LT):
                    emit_proj(lt)

    nc.compile()
    return nc


def make_in_maps(x, w_qkv, b_qkv, w_out, L=L_FULL):
    """Host-side sharding: build the 8 per-core input dicts."""
    # causal mask tiles for diagonal blocks: m[p, j, f] = (128 j + p) <= f
    p = np.arange(128)[:, None, None]
    jj = np.arange(4)[None, :, None]
    f = np.arange(512)[None, None, :]
    masks = ((128 * jj + p) <= f).astype(BF16)

    xT = [np.ascontiguousarray(x[b].T.astype(BF16)) for b in range(B)]
    in_maps = []
    for c in range(N_CORES):
        b, g = divmod(c, TPG)
        h0 = g * HPC  # first global head of this group

        def qcol(h):
            return slice((h0 + h) * D_HEAD, (h0 + h + 1) * D_HEAD)

        def kcol(h):
            return slice(768 + (h0 + h) * D_HEAD, 768 + (h0 + h + 1) * D_HEAD)

        wqkc = np.zeros((D_MODEL, 512), np.float32)
        bqkc = np.zeros((512,), np.float32)
        # chunk0 [q0|q1], chunk1 [k0|k1], chunk2 [q2|-], chunk3 [k2|-]
        for h in range(2):
            wqkc[:, h * 64:(h + 1) * 64] = w_qkv[:, qcol(h)]
            wqkc[:, 128 + h * 64:128 + (h + 1) * 64] = w_qkv[:, kcol(h)]
            bqkc[h * 64:(h + 1) * 64] = b_qkv[qcol(h)]
            bqkc[128 + h * 64:128 + (h + 1) * 64] = b_qkv[kcol(h)]
        wqkc[:, 256:320] = w_qkv[:, qcol(2)]
        bqkc[256:320] = b_qkv[qcol(2)]
        wqkc[:, 384:448] = w_qkv[:, kcol(2)]
        bqkc[384:448] = b_qkv[kcol(2)]

        wv = w_qkv[:, 1536 + h0 * 64:1536 + (h0 + HPC) * 64]
        wo = w_out[h0 * 64:(h0 + HPC) * 64, :]
        wo3 = np.zeros((128, D_MODEL), np.float32)
        wo3[0:64] = wo[128:192]

        in_maps.append({
            "xT": xT[b][:, :L],
            "wqkc": wqkc.astype(BF16),
            "bqkt": np.ascontiguousarray(bqkc.reshape(4, 128).T),
            "wv": np.ascontiguousarray(wv).astype(BF16),
            "wo2": np.ascontiguousarray(wo[0:128]).astype(BF16),
            "wo3": wo3.astype(BF16),
            "masks": masks,
        })
    return in_maps


_NC_CACHE = {}


def _get_nc(L=L_FULL):
    if L not in _NC_CACHE:
        _NC_CACHE[L] = build_nc(L)
    return _NC_CACHE[L]


def run(x, w_qkv, b_qkv, w_out, b_out, L=L_FULL, trace=False):
    nc = _get_nc(L)
    in_maps = make_in_maps(np.asarray(x), np.asarray(w_qkv),
                           np.asarray(b_qkv), np.asarray(w_out), L=L)
    if trace:
        install_ntff()
    res = run_bass_kernel_spmd(nc, in_maps, core_ids=list(range(N_CORES)),
                               trace=trace)
    partials = np.stack([res.results[c]["out"] for c in range(N_CORES)])
    out = partials.reshape(B, TPG, L, D_MODEL).sum(axis=1)
    # the V bias commutes through the attention average (weights sum to 1),
    # so it collapses to a constant row applied after the projection
    bias = np.asarray(b_qkv, np.float32)[1536:] @ np.asarray(w_out, np.float32)
    out = out + (bias + np.asarray(b_out, np.float32))[None, None, :]
    return out.astype(np.float32), res


def kernel(x, w_qkv, b_qkv, w_out, b_out):
    out, _ = run(x, w_qkv, b_qkv, w_out, b_out, L=L_FULL, trace=False)
    return out


# ---- optional NTFF profiling hook (axon images lack antenv.axon_hooks) ----
def install_ntff(so_path="/opt/axon/libaxon_pjrt.so"):
    import contextlib
    import ctypes
    import types

    if "antenv.axon_hooks" in sys.modules:
        return
    holder = {"hook": None}

    def _build():
        if not os.path.exists(so_path):
            return None
        lib = ctypes.CDLL(so_path)
        if not hasattr(lib, "axon_start_nrt_profile"):
            return None
        lib.axon_start_nrt_profile.argtypes = [ctypes.POINTER(ctypes.c_int64),
                                               ctypes.c_size_t]
        lib.axon_start_nrt_profile.restype = ctypes.c_int64
        lib.axon_stop_nrt_profile.argtypes = [ctypes.c_char_p]
        lib.axon_stop_nrt_profile.restype = ctypes.c_int64

        @contextlib.contextmanager
        def _hook(output_dir, device_ids):
            import jax
            jax.devices()
            if device_ids:
                ids = (ctypes.c_int64 * len(device_ids))(*device_ids)
                rc = lib.axon_start_nrt_profile(ids, len(device_ids))
            else:
                rc = lib.axon_start_nrt_profile(None, 0)
            if rc != 0:
                raise RuntimeError(f"axon_start_nrt_profile rc={rc}")
            try:
                yield
            finally:
                n = lib.axon_stop_nrt_profile(str(output_dir).encode())
                print(f"ntff profile: {n} file(s) -> {output_dir}",
                      file=sys.stderr)

        return _hook

    mod = types.ModuleType("antenv.axon_hooks")
    mod.set_axon_ntff_profile_hook = lambda h: holder.__setitem__("hook", h)
    mod.get_axon_ntff_profile_hook = lambda: holder["hook"]
    sys.modules["antenv.axon_hooks"] = mod
    holder["hook"] = _build()


# revision 28
# speedup vs baseline: 2.0850x; 1.0021x over previous
"""Causal self-attention (B=2, L=4096, D=768, H=12) on 8 Trainium2 cores.

Sharding: core c = (b, g) with b = c // 4, g = c % 4. Data-parallel over the
batch, tensor-parallel over heads (3 heads per core). Each core computes its
heads' attention over the full sequence plus its slice of the output
projection (row-parallel); the host sums the 4 partial projections per batch
element and adds b_out.

Device-side design (all matmul operands bf16, fp32 PSUM accumulation):
  - host supplies x[b]^T (768, L); q^T/k^T are produced in [dh, L] layout by
    the projection itself, v in natural [L, dh] layout — no on-device
    transposes anywhere.
  - scores are computed transposed, S^T[lk, lq], 512 lq per PSUM tile; exp
    runs on ScalarE with the 1/sqrt(dh) scale folded in and no
    max-subtraction (scores are ~N(0,1) here, exp is safe in fp32).
  - causal mask = multiply by a 0/1 tile, needed only on the 4 diagonal
    lk-tiles of each lq chunk.
  - PV matmul uses lhsT = [v | ones] so PSUM row 64 accumulates the softmax
    denominator for free; the reciprocal row is partition-broadcast on the
    (otherwise idle) GpSimd engine and applied by one DVE multiply.
  - the PE clock-gate (HAM) only sustains 2.4 GHz when the PE never idles,
    so the qkv projection of the NEXT lq chunk and the output projection of
    the PREVIOUS chunk are emitted as filler work inside the attention
    loop, and PV runs software-pipelined one lk-tile behind the exp.
"""

import os
import sys

sys.path.insert(0, "/opt/trn_rl_repo")

import numpy as np
import ml_dtypes

import concourse.bass as bass  # noqa: F401  (registers AP machinery)
import concourse.mybir as mybir
from concourse import bacc
import concourse.tile as tile
from concourse.bass_utils import run_bass_kernel_spmd

BF16 = ml_dtypes.bfloat16
F32 = mybir.dt.float32
BF = mybir.dt.bfloat16

D_MODEL = 768
N_HEADS = 12
D_HEAD = 64
B = 2
L_FULL = 4096
N_CORES = 8
TPG = 4  # head-groups (tensor-parallel degree per batch element)
HPC = N_HEADS // TPG  # 3 heads per core
DG = HPC * D_HEAD  # 192 feature dims per core
SCALE = 1.0 / np.sqrt(D_HEAD)

DM_CHUNKS = D_MODEL // 128  # 6


def build_nc(L=L_FULL):
    """Build the per-core Bass program (same program for all 8 cores)."""
    LC = L // 512  # lq chunks
    LT = L // 128  # lk / l tiles
    nc = bacc.Bacc("TRN2", target_bir_lowering=False, debug=False,
                   num_devices=N_CORES)

    xT_d = nc.dram_tensor("xT", [D_MODEL, L], BF, kind="ExternalInput").ap()
    wqk_d = nc.dram_tensor("wqkc", [D_MODEL, 512], BF, kind="ExternalInput").ap()
    bqk_d = nc.dram_tensor("bqkt", [128, 4], F32, kind="ExternalInput").ap()
    wv_d = nc.dram_tensor("wv", [D_MODEL, DG], BF, kind="ExternalInput").ap()
    wo2_d = nc.dram_tensor("wo2", [128, D_MODEL], BF, kind="ExternalInput").ap()
    wo3_d = nc.dram_tensor("wo3", [128, D_MODEL], BF, kind="ExternalInput").ap()
    mask_d = nc.dram_tensor("masks", [128, 4, 512], BF, kind="ExternalInput").ap()
    out_d = nc.dram_tensor("out", [L, D_MODEL], F32, kind="ExternalOutput").ap()

    with tile.TileContext(nc) as tc:
        with tc.tile_pool(name="persist", bufs=1) as persist:
            xT_sb = persist.tile([128, DM_CHUNKS, L], BF)
            wqk_sb = persist.tile([128, DM_CHUNKS, 512], BF)
            wv_sb = persist.tile([128, DM_CHUNKS, DG], BF)
            bqk_sb = persist.tile([128, 4], F32)
            wo2_sb = persist.tile([128, D_MODEL], BF)
            wo3_sb = persist.tile([128, D_MODEL], BF)
            mask_sb = persist.tile([128, 4, 512], BF)
            # per-head q^T/k^T, zero-padded to K=128 (rows 64-127 stay zero:
            # a matmul that follows a K=64 matmul pays a ~100ns drain penalty)
            qT = [persist.tile([128, L], BF, name=f"qT{h}") for h in range(HPC)]
            kT = [persist.tile([128, L], BF, name=f"kT{h}") for h in range(HPC)]
            vones = persist.tile([128, LT, HPC * 65], BF)
            attnT01 = persist.tile([128, L], BF)
            attnT2 = persist.tile([128, L], BF)

            # zero-fill the K-padding rows: first-needed on DVE (it is idle at
            # start and must not head-block the projection evacuations), the
            # rest on the otherwise-idle GpSimd, ordered by first use
            nc.vector.memset(kT[0][64:128, :], 0.0)
            nc.vector.memset(qT[0][64:128, :], 0.0)
            nc.vector.memset(vones, 1.0)
            nc.gpsimd.memset(kT[1][64:128, :], 0.0)
            nc.gpsimd.memset(qT[1][64:128, :], 0.0)
            nc.gpsimd.memset(kT[2][64:128, :], 0.0)
            nc.gpsimd.memset(qT[2][64:128, :], 0.0)
            nc.gpsimd.memset(attnT2[64:128, :], 0.0)
            # x^T in lq-ordered strips so chunk 0's projections start early;
            # pair each weight chunk with its strip so fc=0's accumulation
            # chain unblocks as soon as possible
            XSTRIP = min(1024, L)
            for cdm in range(DM_CHUNKS):
                nc.sync.dma_start(out=wqk_sb[:, cdm, :],
                                  in_=wqk_d[cdm * 128:(cdm + 1) * 128, :])
                nc.sync.dma_start(
                    out=xT_sb[:, cdm, 0:XSTRIP],
                    in_=xT_d[cdm * 128:(cdm + 1) * 128, 0:XSTRIP])
            for cdm in range(DM_CHUNKS):
                nc.sync.dma_start(out=wv_sb[:, cdm, :],
                                  in_=wv_d[cdm * 128:(cdm + 1) * 128, :])
            nc.sync.dma_start(out=bqk_sb, in_=bqk_d)
            nc.sync.dma_start(out=wo2_sb, in_=wo2_d)
            nc.sync.dma_start(out=wo3_sb, in_=wo3_d)
            nc.sync.dma_start(out=mask_sb, in_=mask_d)
            for ls in range(1, L // XSTRIP):
                for cdm in range(DM_CHUNKS):
                    nc.sync.dma_start(
                        out=xT_sb[:, cdm, ls * XSTRIP:(ls + 1) * XSTRIP],
                        in_=xT_d[cdm * 128:(cdm + 1) * 128,
                                 ls * XSTRIP:(ls + 1) * XSTRIP])

            # wqkc column chunks: 0=[q0|q1] 1=[k0|k1] 2=[q2|junk] 3=[k2|junk]
            # chunk evacuates into per-head tiles: psum rows 0-63 -> head a
            # rows 0-63, psum rows 64-127 -> head b rows 0-63 (shifted copy)
            qk_dest = [
                (qT[0], qT[1]),
                (kT[0], kT[1]),
                (qT[2], None),
                (kT[2], None),
            ]
            with (
                tc.tile_pool(name="p1psum", bufs=2, space="PSUM") as p1p,
                tc.tile_pool(name="stpsum", bufs=3, space="PSUM") as stp,
                tc.tile_pool(name="pvpsum", bufs=3, space="PSUM") as pvp,
                tc.tile_pool(name="ptpool", bufs=8) as ptp,
                tc.tile_pool(name="rpool", bufs=2) as rp,
                tc.tile_pool(name="outpool", bufs=3) as outp,
            ):
                def emit_qk(fc, lc):
                    dest_a, dest_b = qk_dest[fc]
                    ps = p1p.tile([128, 512], F32, tag="p1",
                                  name=f"psqk{fc}_{lc}")
                    for cdm in range(DM_CHUNKS):
                        nc.tensor.matmul(
                            ps,
                            wqk_sb[:, cdm, fc * 128:(fc + 1) * 128],
                            xT_sb[:, cdm, lc * 512:(lc + 1) * 512],
                            start=(cdm == 0), stop=(cdm == DM_CHUNKS - 1),
                        )
                    # evacuate with fused per-partition (=feature) bias add
                    nc.vector.tensor_scalar_add(
                        dest_a[0:64, lc * 512:(lc + 1) * 512],
                        ps[0:64, :],
                        bqk_sb[0:64, fc:fc + 1],
                    )
                    if dest_b is not None:
                        nc.vector.tensor_scalar_add(
                            dest_b[0:64, lc * 512:(lc + 1) * 512],
                            ps[64:128, :],
                            bqk_sb[64:128, fc:fc + 1],
                        )

                def emit_v(lt):
                    ps = p1p.tile([128, DG], F32, tag="p1", name=f"psv{lt}")
                    for cdm in range(DM_CHUNKS):
                        nc.tensor.matmul(
                            ps,
                            xT_sb[:, cdm, lt * 128:(lt + 1) * 128],
                            wv_sb[:, cdm, :],
                            start=(cdm == 0), stop=(cdm == DM_CHUNKS - 1),
                        )
                    nc.vector.tensor_copy(
                        vones[:, lt, 0:HPC * 65]
                        .rearrange("p (h c) -> p h c", h=HPC)[:, :, 0:64],
                        ps.rearrange("p (h c) -> p h c", h=HPC),
                    )

                def emit_proj(lt):
                    osb = outp.tile([128, D_MODEL], F32, tag="osb",
                                    name=f"osb{lt}")
                    for nh in range(2):
                        po = p1p.tile([128, 384], F32, tag="p1",
                                      name=f"po{lt}_{nh}")
                        nc.tensor.matmul(
                            po,
                            attnT01[:, lt * 128:(lt + 1) * 128],
                            wo2_sb[:, nh * 384:(nh + 1) * 384],
                            start=True, stop=False,
                        )
                        nc.tensor.matmul(
                            po,
                            attnT2[:, lt * 128:(lt + 1) * 128],
                            wo3_sb[:, nh * 384:(nh + 1) * 384],
                            start=False, stop=True,
                        )
                        nc.vector.tensor_copy(osb[:, nh * 384:(nh + 1) * 384], po)
                    nc.sync.dma_start(out=out_d[lt * 128:(lt + 1) * 128, :],
                                      in_=osb)

                def qkv_fillers(lc):
                    fs = [lambda fc=fc: emit_qk(fc, lc) for fc in range(4)]
                    fs += [lambda lt=lt: emit_v(lt)
                           for lt in range(4 * lc, 4 * lc + 4)]
                    return fs

                for f in qkv_fillers(0):
                    f()

                # (attnT tile, destination row base) per head
                norm_dest = [(attnT01, 0), (attnT01, 64), (attnT2, 0)]
                for c in range(LC):
                    nt = 4 * (c + 1)
                    fillers = qkv_fillers(c + 1) if c + 1 < LC else []
                    if c >= 1:
                        fillers += [lambda lt=lt: emit_proj(lt)
                                    for lt in range(4 * (c - 1), 4 * c)]
                    pv_acc = [pvp.tile([65, 512], F32, tag="pvacc",
                                       name=f"pvacc_c{c}h{h}")
                              for h in range(HPC)]
                    prev = []
                    fi = 0
                    for t in range(nt):
                        j = t - 4 * c  # >= 0 on diagonal tiles
                        col0 = 128 * j if j >= 0 else 0
                        cur = []
                        for h in range(HPC):
                            st = stp.tile([128, 512], F32, tag="st",
                                          name=f"st_c{c}t{t}h{h}")
                            nc.tensor.matmul(
                                st[:, col0:],
                                kT[h][:, t * 128:(t + 1) * 128],
                                qT[h][:, c * 512 + col0:(c + 1) * 512],
                            )
                            pt = ptp.tile([128, 512], BF, tag="pt",
                                          name=f"pt_c{c}t{t}h{h}")
                            nc.scalar.activation(
                                pt[:, col0:], st[:, col0:],
                                mybir.ActivationFunctionType.Exp,
                                scale=float(SCALE),
                            )
                            if j >= 0:
                                nc.vector.tensor_mul(
                                    pt[:, col0:], pt[:, col0:],
                                    mask_sb[:, j, col0:],
                                )
                            cur.append((h, pt, col0, t))
                        # PE filler work, spread across the lk-tile loop
                        want = (t + 1) * len(fillers) // nt
                        while fi < want:
                            fillers[fi]()
                            fi += 1
                        # software-pipelined PV: one lk-tile behind
                        for (h, pt0, c0, t0) in prev:
                            nc.tensor.matmul(
                                pv_acc[h][:, c0:],
                                vones[:, t0, h * 65:(h + 1) * 65],
                                pt0[:, c0:],
                                start=(t0 == 0), stop=False,
                            )
                        prev = cur
                    for (h, pt0, c0, t0) in prev:
                        nc.tensor.matmul(
                            pv_acc[h][:, c0:],
                            vones[:, t0, h * 65:(h + 1) * 65],
                            pt0[:, c0:],
                            start=(t0 == 0), stop=True,
                        )
                    prev = []
                    for h in range(HPC):
                        dn = rp.tile([1, 512], F32, tag="dn",
                                     name=f"dn_c{c}h{h}")
                        # partition-shifting copy (psum row 64 -> sbuf row 0);
                        # partition_broadcast only honors a partition-0 source
                        nc.vector.tensor_copy(dn[0:1, :],
                                              pv_acc[h][64:65, :])
                        dnb = rp.tile([64, 512], F32, tag="dnb",
                                      name=f"dnb_c{c}h{h}")
                        nc.gpsimd.partition_broadcast(dnb, dn[0:1, :])
                        rbs = rp.tile([64, 512], F32, tag="rbs",
                                      name=f"rbs_c{c}h{h}")
                        nc.vector.reciprocal_approx_fast(out=rbs, in_=dnb)
                        dt_, r0 = norm_dest[h]
                        nc.vector.tensor_mul(
                            dt_[r0:r0 + 64, c * 512:(c + 1) * 512],
                            pv_acc[h][0:64, :], rbs,
                        )
                for lt in range(4 * (LC - 1), LT):
                    emit_proj(lt)

    nc.compile()
    return nc


def make_in_maps(x, w_qkv, b_qkv, w_out, L=L_FULL):
    """Host-side sharding: build the 8 per-core input dicts."""
    # causal mask tiles for diagonal blocks: m[p, j, f] = (128 j + p) <= f
    p = np.arange(128)[:, None, None]
    jj = np.arange(4)[None, :, None]
    f = np.arange(512)[None, None, :]
    masks = ((128 * jj + p) <= f).astype(BF16)

    xT = [np.ascontiguousarray(x[b].T.astype(BF16)) for b in range(B)]
    in_maps = []
    for c in range(N_CORES):
        b, g = divmod(c, TPG)
        h0 = g * HPC  # first global head of this group

        def qcol(h):
            return slice((h0 + h) * D_HEAD, (h0 + h + 1) * D_HEAD)

        def kcol(h):
            return slice(768 + (h0 + h) * D_HEAD, 768 + (h0 + h + 1) * D_HEAD)

        wqkc = np.zeros((D_MODEL, 512), np.float32)
        bqkc = np.zeros((512,), np.float32)
        # chunk0 [q0|q1], chunk1 [k0|k1], chunk2 [q2|-], chunk3 [k2|-]
        for h in range(2):
            wqkc[:, h * 64:(h + 1) * 64] = w_qkv[:, qcol(h)]
            wqkc[:, 128 + h * 64:128 + (h + 1) * 64] = w_qkv[:, kcol(h)]
            bqkc[h * 64:(h + 1) * 64] = b_qkv[qcol(h)]
            bqkc[128 + h * 64:128 + (h + 1) * 64] = b_qkv[kcol(h)]
        wqkc[:, 256:320] = w_qkv[:, qcol(2)]
        bqkc[256:320] = b_qkv[qcol(2)]
        wqkc[:, 384:448] = w_qkv[:, kcol(2)]
        bqkc[384:448] = b_qkv[kcol(2)]

        wv = w_qkv[:, 1536 + h0 * 64:1536 + (h0 + HPC) * 64]
        wo = w_out[h0 * 64:(h0 + HPC) * 64, :]
        wo3 = np.zeros((128, D_MODEL), np.float32)
        wo3[0:64] = wo[128:192]

        in_maps.append({
            "xT": xT[b][:, :L],
            "wqkc": wqkc.astype(BF16),
            "bqkt": np.ascontiguousarray(bqkc.reshape(4, 128).T),
            "wv": np.ascontiguousarray(wv).astype(BF16),
            "wo2": np.ascontiguousarray(wo[0:128]).astype(BF16),
            "wo3": wo3.astype(BF16),
            "masks": masks,
        })
    return in_maps


_NC_CACHE = {}


def _get_nc(L=L_FULL):
    if L not in _NC_CACHE:
        _NC_CACHE[L] = build_nc(L)
    return _NC_CACHE[L]


def run(x, w_qkv, b_qkv, w_out, b_out, L=L_FULL, trace=False):
    nc = _get_nc(L)
    in_maps = make_in_maps(np.asarray(x), np.asarray(w_qkv),
                           np.asarray(b_qkv), np.asarray(w_out), L=L)
    if trace:
        install_ntff()
    res = run_bass_kernel_spmd(nc, in_maps, core_ids=list(range(N_CORES)),
                               trace=trace)
    partials = np.stack([res.results[c]["out"] for c in range(N_CORES)])
    out = partials.reshape(B, TPG, L, D_MODEL).sum(axis=1)
    # the V bias commutes through the attention average (weights sum to 1),
    # so it collapses to a constant row applied after the projection
    bias = np.asarray(b_qkv, np.float32)[1536:] @ np.asarray(w_out, np.float32)
    out = out + (bias + np.asarray(b_out, np.float32))[None, None, :]
    return out.astype(np.float32), res


def kernel(x, w_qkv, b_qkv, w_out, b_out):
    out, _ = run(x, w_qkv, b_qkv, w_out, b_out, L=L_FULL, trace=False)
    return out


# ---- optional NTFF profiling hook (axon images lack antenv.axon_hooks) ----
def install_ntff(so_path="/opt/axon/libaxon_pjrt.so"):
    import contextlib
    import ctypes
    import types

    if "antenv.axon_hooks" in sys.modules:
        return
    holder = {"hook": None}

    def _build():
        if not os.path.exists(so_path):
            return None
        lib = ctypes.CDLL(so_path)
        if not hasattr(lib, "axon_start_nrt_profile"):
            return None
        lib.axon_start_nrt_profile.argtypes = [ctypes.POINTER(ctypes.c_int64),
                                               ctypes.c_size_t]
        lib.axon_start_nrt_profile.restype = ctypes.c_int64
        lib.axon_stop_nrt_profile.argtypes = [ctypes.c_char_p]
        lib.axon_stop_nrt_profile.restype = ctypes.c_int64

        @contextlib.contextmanager
        def _hook(output_dir, device_ids):
            import jax
            jax.devices()
            if device_ids:
                ids = (ctypes.c_int64 * len(device_ids))(*device_ids)
                rc = lib.axon_start_nrt_profile(ids, len(device_ids))
            else:
                rc = lib.axon_start_nrt_profile(None, 0)
            if rc != 0:
                raise RuntimeError(f"axon_start_nrt_profile rc={rc}")
            try:
                yield
            finally:
                n = lib.axon_stop_nrt_profile(str(output_dir).encode())
                print(f"ntff profile: {n} file(s) -> {output_dir}",
                      file=sys.stderr)

        return _hook

    mod = types.ModuleType("antenv.axon_hooks")
    mod.set_axon_ntff_profile_hook = lambda h: holder.__setitem__("hook", h)
    mod.get_axon_ntff_profile_hook = lambda: holder["hook"]
    sys.modules["antenv.axon_hooks"] = mod
    holder["hook"] = _build()
